# revision 31
# baseline (speedup 1.0000x reference)
"""EfficientViT attention block on 8 TRN2 NeuronCores.

Sharding: 8 cores = 4 images x 2 row-halves (64 rows each + halos).
s=1 cores receive a vertically flipped image + dy-flipped conv weights so the
SPMD program is identical on all cores. The linear-attention kv matrices are
partial sums over each core's own 64 rows, compacted to [128,18] f32 and
combined with a pairwise AllReduce.

Key structure (all intermediates SBUF-resident):
  P1  qkv 3x3 convs (PE, bf16) -> qt[3] [128,70,132]
  P2  dw5x5 fused with grouped pw -> PE block-diag matmuls -> ms[3] [128,66,128]
  P4  per-row DMA transposes + kv/ks PSUM accumulation; diag-compact to [128,18]
  P5  pairwise AllReduce + scatter into att lhsT tiles
  P6/7 fused per 4-row chunk: att matmuls -> recip (DVE) -> PE one-hot
       broadcast -> div (DVE) -> proj matmul -> +ref -> attf [128,66,128]
  P8  mb1 1x1 (PE) + hswish -> h1[6] [128,66,130]
  P9  per 16-row band: dw3 diag matmuls (PE) + hswish + mb3 (PE) + residual
"""
import numpy as np
import ml_dtypes

import concourse.bass as bass
import concourse.bacc as bacc
import concourse.tile as tile
from concourse import mybir
from concourse import bass_utils

F32 = mybir.dt.float32
BF16 = mybir.dt.bfloat16
F8 = mybir.dt.float8e4
Alu = mybir.AluOpType
AF = mybir.ActivationFunctionType
DR = mybir.MatmulPerfMode.DoubleRow
BF = ml_dtypes.bfloat16
F8NP = mybir.dt.np(mybir.dt.float8e4)
S5 = 256.0   # fp8 scale for fused dw5+pw weights
S3 = 384.0   # fp8 scale for dw3 weights (on 6*hswish h1)
S3M = 512.0  # fp8 scale for mb3 weights (on 6*hswish h2)

BN_EPS = 1e-5
NCORES = 8

_CACHE = {}

ATT_SRCS = {0: (0, 1, 2), 1: (2, 3, 4), 2: (4, 5)}  # att9 tile a -> multi tiles

# DoubleRow tap pairings: (dyA, dxA, dyB, dxB, realA). The ifmap AP delta is
# (dyB-dyA)*row_pitch + (dxB-dxA); HW accepts any delta except 1. Pads
# (realA=False) put a zero weight in the A slot.
PAIRS5 = ([(0, dx, 1, dx, True) for dx in range(5)]
          + [(2, dx, 3, dx, True) for dx in range(5)]
          + [(4, 0, 4, 2, True), (4, 1, 4, 3, True), (4, 2, 4, 4, False)])
PAIRS3 = ([(0, dx, 1, dx, True) for dx in range(3)]
          + [(2, 0, 2, 2, True), (0, 1, 2, 1, False)])
NP5 = len(PAIRS5)   # 13
NP3 = len(PAIRS3)   # 5
SC = 512.0          # fp8 scale for qkv conv weights


def q_chan(g, e):
    return 24 * g + e if g < 16 else 384 + 24 * (g - 16) + e


def _row_groups(nrows, gmax=16, base=0):
    out, r = [], 0
    while r < nrows:
        n = min(gmax, nrows - r)
        out.append((base + r, n))
        r += n
    return out


def _subs(nrows, gmax=4):
    return _row_groups(nrows, gmax)


def build_program(sim=False, dbg=None):
    nc = bacc.Bacc("TRN2", target_bir_lowering=False, debug=False,
                   enable_asserts=False, num_devices=NCORES)
    d = {}
    def din(name, shape, dt):
        d[name] = nc.dram_tensor(name, shape, dt, kind="ExternalInput").ap()
    din("xr", [128, 72, 130], BF16)
    din("xr8", [128, 72, 130], F8)
    din("xo8", [128, 72, 130], F8)
    din("wc", [128, 3 * NP3 * 256], F8)
    din("cb", [128, 3], F32)
    din("cm", [128, 3], F32)
    din("dwW", [128, 3 * NP5 * 256], F8)
    din("dw3W", [128, 6 * NP3 * 256], F8)
    din("brd", [128, 96], BF16)
    din("pjw", [128, 3 * 128], BF16)
    din("pjb", [128, 1], F32)
    din("m1w", [128, 6 * 128], BF16)
    din("m1b", [128, 6], F32)
    din("m2b", [128, 6], F32)
    din("m3w8", [128, 3 * 256], F8)
    din("m3b", [128, 1], F32)
    din("idt", [128, 128], BF16)
    d["out"] = nc.dram_tensor("out", [128, 64, 128], F32,
                              kind="ExternalOutput").ap()
    if dbg:
        d["dbg"] = nc.dram_tensor("dbg", [128, 9240], BF16,
                                  kind="ExternalOutput").ap()
    with tile.TileContext(nc) as tc:
        _emit(nc, tc, d, sim=sim, dbg=dbg)
    nc.compile()
    return nc


def _emit(nc, tc, d, sim=False, dbg=None):
    import contextlib
    # pin the ACT table to the one set holding Ln+Exp+Copy+Identity so the
    # per-chunk exp/ln reciprocal never triggers table reloads (1.28us each)
    from concourse.hw_specs import get_activation_tables
    tabs = list(get_activation_tables(nc.m.arch).items())
    idx = next(i for i, (_nm, s) in enumerate(tabs)
               if AF.Ln in s and AF.Exp in s and AF.Copy in s
               and AF.Identity in s)
    ld = mybir.InstLoadActFuncSet(name=nc.get_next_instruction_name(),
                                  ins=[], outs=[], act_func_set_id=idx)
    ld.engine = mybir.EngineType.Activation
    nc.scalar.add_instruction(ld)
    ctx = contextlib.ExitStack()
    with ctx:
        wp = ctx.enter_context(tc.tile_pool(name="wp", bufs=1))
        dram = ctx.enter_context(tc.tile_pool(name="dram", bufs=1, space="DRAM"))
        afp = ctx.enter_context(tc.tile_pool(name="afp", bufs=1))

        def wtile(name, shape, dt):
            t = wp.tile(shape, dt, tag=name, name=name)
            nc.sync.dma_start(out=t, in_=d[name])
            return t

        cb = wtile("cb", [128, 3], F32)
        cm = wtile("cm", [128, 3], F32)
        brd = wtile("brd", [128, 96], BF16)
        pjw = wtile("pjw", [128, 3 * 128], BF16)
        pjb = wtile("pjb", [128, 1], F32)
        m1w = wtile("m1w", [128, 6 * 128], BF16)
        m1b = wtile("m1b", [128, 6], F32)
        m2b = wtile("m2b", [128, 6], F32)
        m3b = wtile("m3b", [128, 1], F32)
        ident = wtile("idt", [128, 128], BF16)
        # v-gather staging with a built-in ones column (ks = kv @ ones)
        vcs = []
        for i in range(2):
            vct = wp.tile([128, 2, 16, 9], BF16, tag=f"vc{i}", name=f"vc{i}")
            nc.vector.memset(vct[:, :, :, 8:9], 1.0)
            vcs.append(vct)
        comp = wp.tile([128, 288], BF16, tag="comp", name="comp")
        # att lhsT tiles (bf16); the bf16 AllReduce result scatters into
        # them directly
        lhsT_att = {}
        for a in ATT_SRCS:
            for S in ATT_SRCS[a]:
                bt = wp.tile([128, 108], BF16, tag=f"lat{a}_{S}", name=f"lat{a}_{S}")
                nc.vector.memset(bt, 0.0)
                lhsT_att[(a, S)] = bt

        cc_in = dram.tile([128, 288], BF16, tag="cc_in", name="cc_in")
        cc_out = dram.tile([128, 288], BF16, tag="cc_out", name="cc_out")
        cc_win = dram.tile([128, 4], F32, tag="cc_win", name="cc_win")
        cc_wout = dram.tile([128, 4], F32, tag="cc_wout", name="cc_wout")
        # warm-up collective: absorbs the one-time CC startup latency while
        # P1 computes
        nc.vector.memset(warm := wp.tile([128, 4], F32, tag="warm",
                                         name="warm"), 0.0)
        nc.sync.dma_start(out=cc_win[:], in_=warm)
        if not sim:
            nc.gpsimd.collective_compute(
                "AllReduce", Alu.add,
                replica_groups=[[0, 1], [2, 3], [4, 5], [6, 7]],
                ins=[cc_win.opt()], outs=[cc_wout.opt()])

        attf = afp.tile([128, 66, 128], BF16, tag="attf", name="attf")
        # xr doubles as the ref-residual source (rows lr-1..64 = idx 3..69)
        xr = afp.tile([128, 72, 130], BF16, tag="xr", name="xr")
        nc.sync.dma_start(out=xr[:, 0:36, :], in_=d["xr"][:, 0:36, :])
        nc.scalar.dma_start(out=xr[:, 36:72, :], in_=d["xr"][:, 36:72, :])

        with tc.tile_pool(name="qp", bufs=1) as qp, \
             tc.tile_pool(name="msp", bufs=1) as msp:
            dwstack = contextlib.ExitStack()
            dwp = dwstack.enter_context(tc.tile_pool(name="dwp", bufs=1))
            dwW = dwp.tile([128, 3 * NP5 * 256], F8, tag="dwW", name="dwW")
            nc.sync.dma_start(out=dwW[:, 0:4992], in_=d["dwW"][:, 0:4992])
            nc.scalar.dma_start(out=dwW[:, 4992:], in_=d["dwW"][:, 4992:])

            # ---- P1: qkv convs (bf16 out + fp8 copy for dw5) ----
            qkv = []
            qkv8 = []
            with tc.tile_pool(name="xp", bufs=1) as xp, \
                 tc.tile_pool(name="cps", bufs=2, space="PSUM") as cps:
                xr8 = xp.tile([128, 72, 130], F8, tag="xr8", name="xr8")
                nc.sync.dma_start(out=xr8[:, 0:36, :], in_=d["xr8"][:, 0:36, :])
                nc.scalar.dma_start(out=xr8[:, 36:72, :],
                                    in_=d["xr8"][:, 36:72, :])
                xo8 = xp.tile([128, 72, 130], F8, tag="xo8", name="xo8")
                nc.sync.dma_start(out=xo8[:, 0:36, :], in_=d["xo8"][:, 0:36, :])
                nc.scalar.dma_start(out=xo8[:, 36:72, :],
                                    in_=d["xo8"][:, 36:72, :])
                wc = xp.tile([128, 3 * NP3 * 256], F8, tag="wc", name="wc")
                nc.sync.dma_start(out=wc, in_=d["wc"])
                for j in range(3):
                    # bf16 tile: rows = lr -1..64 (66) + 2 zero pad rows so the
                    # att chunks are uniformly 4 rows wide, cols = x 0..128
                    # (no halo; only the fp8 copy needs conv halos for dw5)
                    qt = qp.tile([128, 66, 128], BF16, tag=f"q{j}", name=f"q{j}")
                    nc.vector.memset(qt[:, 0:1, :], 0.0)
                    qt8 = dwp.tile([128, 70, 132], F8, tag=f"q8{j}",
                                   name=f"q8{j}")
                    nc.vector.memset(qt8[:, 0:3, :], 0.0)
                    nc.vector.memset(qt8[:, 3:70, 0:2], 0.0)
                    nc.vector.memset(qt8[:, 3:70, 130:132], 0.0)
                    src = xr8 if j == 0 else xo8
                    for (r0, nr) in _row_groups(67, base=3):
                        ps = cps.tile([128, 16 * 128], F32, tag="cps", name="cps")
                        psv = ps[:, 0:nr * 128].rearrange("p (r w) -> p r w", w=128)
                        for (sr, sn) in _subs(nr):
                            for p_, (dyA, dxA, dyB, dxB, _ra) in enumerate(
                                    PAIRS3):
                                dlt = (dyB - dyA) * 130 + (dxB - dxA)
                                win = src[:, r0 + sr + dyA:
                                          r0 + sr + dyA + sn, dxA:dxA + 128]
                                rhs = bass.AP(
                                    tensor=win.tensor, offset=win.offset,
                                    ap=[list(win.ap[0]), [dlt, 2],
                                        list(win.ap[1]), list(win.ap[2])])
                                w8 = wc[:, (j * NP3 + p_) * 256:
                                        (j * NP3 + p_) * 256 + 256].rearrange(
                                    "p (k m) -> p k m", k=2)
                                nc.tensor.matmul(
                                    psv[:, sr:sr + sn, :], w8, rhs,
                                    start=(p_ == 0), stop=(p_ == NP3 - 1),
                                    perf_mode=DR)
                        cnt = min(nr, 68 - r0)   # qt rows = old idx 2..67
                        nc.scalar.activation(
                            out=qt[:, r0 - 2:r0 - 2 + cnt, :],
                            in_=psv[:, 0:cnt, :],
                            func=AF.Identity, bias=cb[:, j:j + 1],
                            scale=1.0 / SC)
                        with nc.allow_low_precision(reason="fp8 dw5 input"):
                            nc.vector.tensor_scalar(
                                out=qt8[:, r0:r0 + nr, 2:130],
                                in0=psv[:, 0:nr, :],
                                scalar1=1.0 / SC, scalar2=cb[:, j:j + 1],
                                op0=Alu.mult, op1=Alu.add)
                    qkv.append(qt)
                    qkv8.append(qt8)
            # relu-mask bf16 qkv right away (dw5 uses the raw fp8 copy)
            for t in range(3):
                nc.vector.tensor_scalar_max(out=qkv[t], in0=qkv[t],
                                            scalar1=cm[:, t:t + 1])

            # ---- P2+P4 fused: dw5(fp8 DoubleRow) chunks interleaved with
            #      per-row transposes + kv matmuls ----
            ms = [msp.tile([128, 66, 128], BF16, tag=f"ms{t}", name=f"ms{t}")
                  for t in range(3)]
            with tc.tile_pool(name="dps", bufs=2, space="PSUM") as dps, \
                 tc.tile_pool(name="kps", bufs=1, space="PSUM") as kps, \
                 tc.tile_pool(name="tpp", bufs=4, space="PSUM") as tpp, \
                 tc.tile_pool(name="mtp", bufs=8) as mtp:
                kvT = [kps.tile([128, 144], F32, tag=f"kvT{h}", name=f"kvT{h}")
                       for h in range(2)]

                def kv_row(r):
                    # all 6 row transposes on PE (identity transpose-matmul);
                    # psum->sbuf copies alternate DVE/ACT
                    mT = mtp.tile([128, 768], BF16, tag="mT", name="mT")
                    for i, (src, dst0) in enumerate(
                            [(qkv[0][:, 1 + r, :], 0),
                             (ms[0][:, 1 + r, :], 384),
                             (qkv[1][:, 1 + r, :], 128),
                             (ms[1][:, 1 + r, :], 512),
                             (qkv[2][:, 1 + r, :], 256),
                             (ms[2][:, 1 + r, :], 640)]):
                        tp = tpp.tile([128, 128], BF16, tag="tp", name="tp")
                        nc.tensor.transpose(tp, src, ident)
                        if (i + r) % 2 == 0:
                            nc.vector.tensor_copy(
                                out=mT[:, dst0:dst0 + 128], in_=tp)
                        else:
                            nc.scalar.activation(
                                out=mT[:, dst0:dst0 + 128], in_=tp,
                                func=AF.Copy)
                    mg = mT.rearrange("p (g c) -> p g c", c=24)
                    kc = mtp.tile([128, 256], BF16, tag="kc", name="kc")
                    nc.vector.tensor_copy(
                        out=kc.rearrange("p (g e) -> p g e", e=8),
                        in_=mg[:, :, 8:16])
                    vc = vcs[r % 2]
                    nc.vector.tensor_copy(
                        out=vc.rearrange("p h g e -> p (h g) e")[:, :, 0:8],
                        in_=mg[:, :, 16:24])
                    for h in range(2):
                        kcols = kc[:, 128 * h:128 * h + 128]
                        nc.tensor.matmul(kvT[h], kcols, vc[:, h, :, :],
                                         start=(r == 0), stop=(r == 63))

                rdone = 0
                for ci, (c0, cn) in enumerate(_subs(66)):
                    for t in range(3):
                        ps = dps.tile([128, 512], F32, tag="dps", name="dps")
                        q8 = qkv8[t]
                        for p_, (dyA, dxA, dyB, dxB, _ra) in enumerate(
                                PAIRS5):
                            dlt = (dyB - dyA) * 132 + (dxB - dxA)
                            win = q8[:, c0 + dyA:c0 + dyA + cn,
                                     dxA:dxA + 128]
                            rhs = bass.AP(
                                tensor=win.tensor, offset=win.offset,
                                ap=[list(win.ap[0]), [dlt, 2],
                                    list(win.ap[1]), list(win.ap[2])])
                            w8 = dwW[:, (t * NP5 + p_) * 256:
                                     (t * NP5 + p_) * 256 + 256].rearrange(
                                "p (k m) -> p k m", k=2)
                            nc.tensor.matmul(ps[:, 0:cn * 128], w8, rhs,
                                             start=(p_ == 0),
                                             stop=(p_ == NP5 - 1),
                                             perf_mode=DR)
                        nc.scalar.activation(
                            out=ms[t][:, c0:c0 + cn, :],
                            in_=ps[:, 0:cn * 128].rearrange(
                                "p (r w) -> p r w", w=128),
                            func=AF.Copy, scale=1.0 / S5)
                        nc.vector.tensor_scalar_max(
                            out=ms[t][:, c0:c0 + cn, :],
                            in0=ms[t][:, c0:c0 + cn, :],
                            scalar1=cm[:, t:t + 1])
                    lim = min(64, c0 + cn - 1)
                    while rdone < lim:
                        kv_row(rdone)
                        rdone += 1
                while rdone < 64:
                    kv_row(rdone)
                    rdone += 1
                with nc.allow_low_precision(reason="bf16 kv exchange"):
                    for h in range(2):
                        nc.vector.tensor_copy(
                            out=comp[:, 144 * h:144 * h + 144], in_=kvT[h])
            dwstack.close()
            if dbg and dbg.startswith("ms"):
                ti = int(dbg[2])
                nc.sync.dma_start(
                    out=d["dbg"][:, 0:66 * 128].rearrange(
                        "p (r w) -> p r w", w=128), in_=ms[ti])
            if dbg and dbg.startswith("qm"):
                ti = int(dbg[2])
                nc.sync.dma_start(
                    out=d["dbg"][:, 0:66 * 128].rearrange(
                        "p (r w) -> p r w", w=128), in_=qkv[ti])

            # ---- P5: AllReduce + scatter ----
            nc.sync.dma_start(out=cc_in[:], in_=comp)
            if sim:
                nc.sync.dma_start(out=cc_out[:], in_=cc_in[:])
            else:
                nc.gpsimd.collective_compute(
                    "AllReduce", Alu.add,
                    replica_groups=[[0, 1], [2, 3], [4, 5], [6, 7]],
                    ins=[cc_in.opt()], outs=[cc_out.opt()])
            for g in range(32):
                a, gl9 = g // 12, g % 12
                h, gl = g // 16, g % 16
                S, row0 = q_chan(g, 0) // 128, q_chan(g, 0) % 128
                L = lhsT_att[(a, S)].rearrange("p (dd gl) -> p dd gl", gl=12)
                eng = (nc.sync, nc.scalar, nc.gpsimd)[g % 3]
                eng.dma_start(
                    out=L[row0:row0 + 8, 0:9, gl9:gl9 + 1],
                    in_=cc_out[8 * gl:8 * gl + 8,
                               144 * h + 9 * gl:144 * h + 9 * gl + 9])
            # ---- P6+P7 fused: att9 + division + proj + residual per chunk ----
            # denominator reciprocal on ACT via exp(-ln(x + eps)): the exp/ln
            # table set also holds copy/identity/relu so no table reloads; the
            # DVE 8-pass iterative reciprocal is gone entirely.
            with tc.tile_pool(name="aps", bufs=2, space="PSUM") as aps, \
                 tc.tile_pool(name="bps", bufs=1, space="PSUM") as bpsp, \
                 tc.tile_pool(name="jps", bufs=1, space="PSUM") as jpsp, \
                 tc.tile_pool(name="dnp", bufs=3) as dnp:
                epsb = dnp.tile([128, 1], F32, tag="epsb", name="epsb",
                                bufs=1)
                nc.vector.memset(epsb, 1e-15)

                def att_rhs(S, c0, cn):
                    if S < 3:
                        return qkv[S][:, c0:c0 + cn, :]
                    return ms[S - 3][:, c0:c0 + cn, :]

                for ci, (c0, cn) in enumerate(_subs(66)):
                    cw = cn * 128
                    psl = aps.tile([108, 3 * 512], F32, tag="aps", name="aps")
                    for a in range(3):
                        srcs = ATT_SRCS[a]
                        for i, S in enumerate(srcs):
                            nc.tensor.matmul(psl[:, a * 512:a * 512 + cw],
                                             lhsT_att[(a, S)],
                                             att_rhs(S, c0, cn), start=(i == 0),
                                             stop=(i == len(srcs) - 1))
                    lnden = dnp.tile([12, 3 * 512], F32, tag="lnden",
                                     name="lnden")
                    nc.scalar.activation(out=lnden, in_=psl[96:108, :],
                                         func=AF.Ln, bias=epsb[0:12, 0:1],
                                         scale=1.0)
                    rden = dnp.tile([12, 3 * 512], BF16, tag="rden",
                                    name="rden")
                    with nc.allow_low_precision(reason="den recip to bf16"):
                        nc.scalar.activation(out=rden, in_=lnden, func=AF.Exp,
                                             scale=-1.0)
                    jp = jpsp.tile([128, 512], F32, tag="jps", name="jps")
                    for a in range(3):
                        bp = bpsp.tile([96, 512], F32, tag="bps", name="bps")
                        nc.tensor.matmul(bp, brd[0:12, :],
                                         rden[:, a * 512:a * 512 + 512],
                                         start=True, stop=True)
                        recb = dnp.tile([96, 512], BF16, tag="recb",
                                        name="recb")
                        with nc.allow_low_precision(reason="recb bf16"):
                            if a == 1:
                                nc.scalar.activation(out=recb, in_=bp,
                                                     func=AF.Copy)
                            else:
                                nc.vector.tensor_copy(out=recb, in_=bp)
                        attc = dnp.tile([96, 512], BF16, tag="attc",
                                        name="attc")
                        with nc.allow_low_precision(reason="att to bf16"):
                            nc.vector.tensor_mul(
                                out=attc, in0=psl[0:96, a * 512:a * 512 + 512],
                                in1=recb)
                        nc.tensor.matmul(jp,
                                         pjw[0:96, a * 128:a * 128 + 128],
                                         attc,
                                         start=(a == 0), stop=(a == 2))
                    aB = dnp.tile([128, 512], BF16, tag="aB", name="aB")
                    nc.scalar.activation(out=aB, in_=jp,
                                         func=AF.Identity, bias=pjb[:, 0:1],
                                         scale=1.0)
                    nc.vector.tensor_add(
                        out=attf[:, c0:c0 + cn, :],
                        in0=aB[:, 0:cw].rearrange("p (r w) -> p r w", w=128),
                        in1=xr[:, c0 + 3:c0 + 3 + cn, 1:129])

        if dbg == "attf":
            nc.sync.dma_start(
                out=d["dbg"][:, 0:66 * 128].rearrange(
                    "p (r w) -> p r w", w=128), in_=attf)
        # ---- P8: mb1 + hswish -> h1 fp8 (SBUF resident) ----
        with tc.tile_pool(name="h1p", bufs=1) as h1p, \
             tc.tile_pool(name="lwp", bufs=1) as lwp:
            dw3W = lwp.tile([128, 6 * NP3 * 256], F8, tag="dw3W", name="dw3W")
            nc.sync.dma_start(out=dw3W, in_=d["dw3W"])
            h1 = []
            for t in range(6):
                ht = h1p.tile([128, 66, 130], F8, tag=f"h1{t}", name=f"h1{t}")
                nc.vector.memset(ht[:, 0:1, :], 0.0)
                nc.vector.memset(ht[:, 1:66, 0:1], 0.0)
                nc.vector.memset(ht[:, 1:66, 129:130], 0.0)
                h1.append(ht)
            with tc.tile_pool(name="m1ps", bufs=1, space="PSUM") as m1ps, \
                 tc.tile_pool(name="hwp", bufs=2) as hwp:
                for (r0, nr) in _row_groups(65, base=1):
                    for t in range(6):
                        ps = m1ps.tile([128, 16 * 128], F32, tag="m1ps", name="m1ps")
                        for (sr, sn) in _subs(nr):
                            nc.tensor.matmul(
                                ps[:, sr * 128:(sr + sn) * 128],
                                m1w[:, t * 128:t * 128 + 128],
                                attf[:, r0 + sr:r0 + sr + sn, :],
                                start=True, stop=True)
                        pw_ = ps[:, 0:nr * 128]
                        # 6*hswish(x) = min(relu(x+3),6)*x in 2 DVE ops; the
                        # 1/6 is folded into the host dw3 weights
                        th = hwp.tile([128, 16 * 128], BF16, tag="th", name="th")
                        xh = hwp.tile([128, 16 * 128], BF16, tag="xh", name="xh")
                        nc.scalar.activation(out=xh[:, 0:nr * 128], in_=pw_,
                                             func=AF.Identity,
                                             bias=m1b[:, t:t + 1], scale=1.0)
                        nc.vector.tensor_scalar(
                            out=th[:, 0:nr * 128], in0=xh[:, 0:nr * 128],
                            scalar1=3.0, scalar2=0.0,
                            op0=Alu.add, op1=Alu.max)
                        if t % 2 == 0:
                            with nc.allow_low_precision(reason="h1 fp8"):
                                nc.vector.scalar_tensor_tensor(
                                    out=h1[t][:, r0:r0 + nr, 1:129],
                                    in0=th[:, 0:nr * 128].rearrange(
                                        "p (r w) -> p r w", w=128),
                                    scalar=6.0,
                                    in1=xh[:, 0:nr * 128].rearrange(
                                        "p (r w) -> p r w", w=128),
                                    op0=Alu.min, op1=Alu.mult)
                        else:
                            hb = hwp.tile([128, 16 * 128], BF16, tag="hb",
                                          name="hb")
                            nc.vector.scalar_tensor_tensor(
                                out=hb[:, 0:nr * 128], in0=th[:, 0:nr * 128],
                                scalar=6.0, in1=xh[:, 0:nr * 128],
                                op0=Alu.min, op1=Alu.mult)
                            with nc.allow_low_precision(reason="h1 fp8"):
                                nc.scalar.activation(
                                    out=h1[t][:, r0:r0 + nr, 1:129],
                                    in_=hb[:, 0:nr * 128].rearrange(
                                        "p (r w) -> p r w", w=128),
                                    func=AF.Copy)

            # ---- P9: dw3 (fp8 DoubleRow) + hswish + mb3 + final adds ----
            with tc.tile_pool(name="d3ps", bufs=1, space="PSUM") as d3ps, \
                 tc.tile_pool(name="m3ps", bufs=1, space="PSUM") as m3ps, \
                 tc.tile_pool(name="h2p", bufs=2) as h2p, \
                 tc.tile_pool(name="osp", bufs=2) as osp:
                m3w8 = lwp.tile([128, 3 * 256], F8, tag="m3w8", name="m3w8")
                nc.scalar.dma_start(out=m3w8, in_=d["m3w8"])
                for q in range(8):   # 8-row half-bands, out rows lr 8q..8q+8
                    ps = m3ps.tile([128, 8 * 128], F32, tag="m3ps", name="m3ps")
                    h2s = h2p.tile([128, 6, 8 * 128], F8, tag="h2s",
                                   name="h2s")
                    for t in range(6):
                        dp = d3ps.tile([128, 8 * 128], F32, tag="d3ps", name="d3ps")
                        hh = h1[t]
                        for (sr, sn) in _subs(8):
                            for p_, (dyA, dxA, dyB, dxB, _ra) in enumerate(
                                    PAIRS3):
                                dlt = (dyB - dyA) * 130 + (dxB - dxA)
                                win = hh[:, 8 * q + sr + dyA:
                                         8 * q + sr + dyA + sn,
                                         dxA:dxA + 128]
                                rhs = bass.AP(
                                    tensor=win.tensor, offset=win.offset,
                                    ap=[list(win.ap[0]), [dlt, 2],
                                        list(win.ap[1]), list(win.ap[2])])
                                w8 = dw3W[:, (t * NP3 + p_) * 256:
                                          (t * NP3 + p_) * 256 + 256].rearrange(
                                    "p (k m) -> p k m", k=2)
                                nc.tensor.matmul(
                                    dp[:, sr * 128:(sr + sn) * 128], w8, rhs,
                                    start=(p_ == 0), stop=(p_ == NP3 - 1),
                                    perf_mode=DR)
                        th = h2p.tile([128, 8 * 128], BF16, tag="th2", name="th2")
                        xh = h2p.tile([128, 8 * 128], BF16, tag="xh2", name="xh2")
                        nc.scalar.activation(out=xh, in_=dp, func=AF.Identity,
                                             bias=m2b[:, t:t + 1], scale=1.0 / S3)
                        nc.vector.tensor_scalar(
                            out=th, in0=xh, scalar1=3.0, scalar2=0.0,
                            op0=Alu.add, op1=Alu.max)
                        with nc.allow_low_precision(reason="h2 fp8"):
                            nc.vector.scalar_tensor_tensor(
                                out=h2s[:, t, :], in0=th, scalar=6.0, in1=xh,
                                op0=Alu.min, op1=Alu.mult)
                    for pr in range(3):
                        for (sr, sn) in _subs(8):
                            win = h2s[:, 2 * pr, sr * 128:(sr + sn) * 128]
                            rhs = bass.AP(
                                tensor=win.tensor, offset=win.offset,
                                ap=[list(win.ap[0]), [1024, 2],
                                    list(win.ap[1])])
                            w8 = m3w8[:, pr * 256:pr * 256 + 256].rearrange(
                                "p (k m) -> p k m", k=2)
                            nc.tensor.matmul(
                                ps[:, sr * 128:(sr + sn) * 128], w8, rhs,
                                start=(pr == 0), stop=(pr == 2),
                                perf_mode=DR)
                    o1 = osp.tile([128, 8 * 128], F32, tag="o1", name="o1")
                    nc.scalar.activation(out=o1, in_=ps, func=AF.Identity,
                                         bias=m3b[:, 0:1], scale=1.0 / S3M)
                    nc.vector.tensor_add(
                        out=o1, in0=o1,
                        in1=attf[:, 8 * q + 1:8 * q + 9, :].rearrange(
                            "p r w -> p (r w)"))
                    nc.sync.dma_start(out=d["out"][:, 8 * q:8 * q + 8, :],
                                      in_=o1.rearrange("p (r w) -> p r w", w=128))


# ====================== host side ======================

def _prep_shared(inp):
    f32 = np.float32
    out = {}
    pw = inp["agg_pw_w"][:, :, 0, 0]          # [384, 8]
    w5 = inp["agg_dw_w"][:, 0, :, :]          # [384, 5, 5]
    w3 = inp["mb2_w"][:, 0, :, :]             # [768, 3, 3]
    for s in (0, 1):
        w = {}
        wc = np.zeros((128, 3 * NP3 * 256), f32)
        for j, cw in enumerate((inp["wq"], inp["wk"], inp["wv"])):
            def c_mat(dy, dx):
                dyy = 2 - dy if s == 1 else dy
                return cw[:, :, dyy, dx].T * SC
            for p_, (dyA, dxA, dyB, dxB, realA) in enumerate(PAIRS3):
                k = (j * NP3 + p_) * 256
                if realA:
                    wc[:, k:k + 128] = c_mat(dyA, dxA)
                wc[:, k + 128:k + 256] = c_mat(dyB, dxB)
        w["wc"] = wc.astype(F8NP)
        w["cb"] = np.stack([inp["bq"], inp["bk"], inp["bv"]], 1).astype(f32)
        m = np.arange(384)
        w["cm"] = np.where((m % 24) < 16, 0.0, -1e9).astype(f32).reshape(3, 128).T.copy()
        # fused dw5x5 + grouped pw block-diag weights, fp8 DoubleRow pairs
        def dw5_mat(t, dy, dx):
            dyy = 4 - dy if s == 1 else dy
            M = np.zeros((128, 128), f32)
            for b in range(16):
                i0 = 8 * b
                blk = (w5[128 * t + i0:128 * t + i0 + 8, dyy, dx][:, None]
                       * pw[128 * t + i0:128 * t + i0 + 8, :].T)
                M[i0:i0 + 8, i0:i0 + 8] = blk
            return M * S5
        dwW = np.zeros((128, 3 * NP5 * 256), f32)
        for t in range(3):
            for p_, (dyA, dxA, dyB, dxB, realA) in enumerate(PAIRS5):
                k = (t * NP5 + p_) * 256
                if realA:
                    dwW[:, k:k + 128] = dw5_mat(t, dyA, dxA)
                dwW[:, k + 128:k + 256] = dw5_mat(t, dyB, dxB)
        w["dwW"] = dwW.astype(F8NP)
        # dw3 diagonal weights, fp8 DoubleRow vertical pairs
        def dw3_mat(t, dy, dx):
            dyy = 2 - dy if s == 1 else dy
            M = np.zeros((128, 128), f32)
            M[np.arange(128), np.arange(128)] = \
                w3[128 * t:128 * t + 128, dyy, dx] * (S3 / 6.0)
            return M
        dw3W = np.zeros((128, 6 * NP3 * 256), f32)
        for t in range(6):
            for p_, (dyA, dxA, dyB, dxB, realA) in enumerate(PAIRS3):
                k = (t * NP3 + p_) * 256
                if realA:
                    dw3W[:, k:k + 128] = dw3_mat(t, dyA, dxA)
                dw3W[:, k + 128:k + 256] = dw3_mat(t, dyB, dxB)
        w["dw3W"] = dw3W.astype(F8NP)
        # one-hot broadcast for denominators: rows 96+gl9 -> out col o (gl9=o%12)
        brd = np.zeros((128, 96), f32)
        o = np.arange(96)
        brd[o % 12, o] = 1.0
        w["brd"] = brd.astype(BF)
        s1 = inp["bn1_g"] / np.sqrt(inp["bn1_v"] + BN_EPS)
        b1 = inp["bn1_b"] - inp["bn1_m"] * s1
        Wp = inp["attn_proj_w"][:, :, 0, 0] * s1[:, None]
        pjw = np.zeros((128, 3 * 128), f32)
        for g in range(32):
            a, gl9 = g // 12, g % 12
            for dd in range(8):
                pjw[12 * dd + gl9, a * 128:a * 128 + 128] = Wp[:, 8 * g + dd]
        w["pjw"] = pjw.astype(BF)
        w["pjb"] = b1.reshape(128, 1).astype(f32)
        m1w = np.zeros((128, 6 * 128), f32)
        for t in range(6):
            m1w[:, t * 128:t * 128 + 128] = inp["mb1_w"][128 * t:128 * t + 128, :, 0, 0].T
        w["m1w"] = m1w.astype(BF)
        w["m1b"] = inp["mb1_b"].reshape(6, 128).T.copy().astype(f32)
        w["m2b"] = inp["mb2_b"].reshape(6, 128).T.copy().astype(f32)
        s2 = inp["bn2_g"] / np.sqrt(inp["bn2_v"] + BN_EPS)
        b2 = inp["bn2_b"] - inp["bn2_m"] * s2
        W3 = inp["mb3_w"][:, :, 0, 0] * s2[:, None]
        m3w8 = np.zeros((128, 3 * 256), f32)
        for t in range(6):
            half = 128 * (t % 2)
            k = (t // 2) * 256 + half
            m3w8[:, k:k + 128] = \
                W3[:, 128 * t:128 * t + 128].T * (S3M / 6.0)
        w["m3w8"] = m3w8.astype(F8NP)
        w["m3b"] = b2.reshape(128, 1).astype(f32)
        w["idt"] = np.eye(128, dtype=f32).astype(BF)
        out[s] = w
    return out


def _prep_core(inp, b, s):
    f32 = np.float32
    ref = inp["ref_features"][b]
    oth = inp["other_features"][b]
    if s == 1:
        ref = ref[:, ::-1, :]
        oth = oth[:, ::-1, :]
    xr = np.zeros((128, 72, 130), f32)
    xo = np.zeros((128, 72, 130), f32)
    xr[:, 4:72, 1:129] = ref[:, 0:68, :]
    xo[:, 4:72, 1:129] = oth[:, 0:68, :]
    return {"xr": xr.astype(BF), "xr8": xr.astype(F8NP),
            "xo8": xo.astype(F8NP)}


def make_in_maps(inp):
    ws = _prep_shared(inp)
    in_maps = []
    for c in range(NCORES):
        b, s = c // 2, c % 2
        m = dict(ws[s])
        m.update(_prep_core(inp, b, s))
        in_maps.append(m)
    return in_maps


def kernel(**inputs):
    inp = {k: np.asarray(v) for k, v in inputs.items()}
    if "nc" not in _CACHE:
        _CACHE["nc"] = build_program()
    nc = _CACHE["nc"]
    in_maps = make_in_maps(inp)
    res = bass_utils.run_bass_kernel_spmd(nc, in_maps,
                                          core_ids=list(range(NCORES)))
    out = np.zeros((4, 128, 128, 128), np.float32)
    for c in range(NCORES):
        b, s = c // 2, c % 2
        o = res.results[c]["out"]
        if s == 1:
            o = o[:, ::-1, :]
        out[b, :, 64 * s:64 * s + 64, :] = o
    return out



# revision 33
# speedup vs baseline: 1.0264x; 1.0264x over previous
"""EfficientViT attention block on 8 TRN2 NeuronCores.

Sharding: 8 cores = 4 images x 2 row-halves (64 rows each + halos).
s=1 cores receive a vertically flipped image + dy-flipped conv weights so the
SPMD program is identical on all cores. The linear-attention kv matrices are
partial sums over each core's own 64 rows, compacted to [128,18] f32 and
combined with a pairwise AllReduce.

Key structure (all intermediates SBUF-resident):
  P1  qkv 3x3 convs (PE, bf16) -> qt[3] [128,70,132]
  P2  dw5x5 fused with grouped pw -> PE block-diag matmuls -> ms[3] [128,66,128]
  P4  per-row DMA transposes + kv/ks PSUM accumulation; diag-compact to [128,18]
  P5  pairwise AllReduce + scatter into att lhsT tiles
  P6/7 fused per 4-row chunk: att matmuls -> recip (DVE) -> PE one-hot
       broadcast -> div (DVE) -> proj matmul -> +ref -> attf [128,66,128]
  P8  mb1 1x1 (PE) + hswish -> h1[6] [128,66,130]
  P9  per 16-row band: dw3 diag matmuls (PE) + hswish + mb3 (PE) + residual
"""
import numpy as np
import ml_dtypes

import concourse.bass as bass
import concourse.bacc as bacc
import concourse.tile as tile
from concourse import mybir
from concourse import bass_utils

F32 = mybir.dt.float32
BF16 = mybir.dt.bfloat16
F8 = mybir.dt.float8e4
Alu = mybir.AluOpType
AF = mybir.ActivationFunctionType
DR = mybir.MatmulPerfMode.DoubleRow
BF = ml_dtypes.bfloat16
F8NP = mybir.dt.np(mybir.dt.float8e4)
S5 = 256.0   # fp8 scale for fused dw5+pw weights
S3 = 384.0   # fp8 scale for dw3 weights (on 6*hswish h1)
S3M = 512.0  # fp8 scale for mb3 weights (on 6*hswish h2)

BN_EPS = 1e-5
NCORES = 8

_CACHE = {}

ATT_SRCS = {0: (0, 1, 2), 1: (2, 3, 4), 2: (4, 5)}  # att9 tile a -> multi tiles

# DoubleRow tap pairings: (dyA, dxA, dyB, dxB, realA). The ifmap AP delta is
# (dyB-dyA)*row_pitch + (dxB-dxA); HW accepts any delta except 1. Pads
# (realA=False) put a zero weight in the A slot.
PAIRS5 = ([(0, dx, 1, dx, True) for dx in range(5)]
          + [(2, dx, 3, dx, True) for dx in range(5)]
          + [(4, 0, 4, 2, True), (4, 1, 4, 3, True), (4, 2, 4, 4, False)])
PAIRS3 = ([(0, dx, 1, dx, True) for dx in range(3)]
          + [(2, 0, 2, 2, True), (0, 1, 2, 1, False)])
NP5 = len(PAIRS5)   # 13
NP3 = len(PAIRS3)   # 5
SC = 512.0          # fp8 scale for qkv conv weights


def q_chan(g, e):
    return 24 * g + e if g < 16 else 384 + 24 * (g - 16) + e


def _row_groups(nrows, gmax=16, base=0):
    out, r = [], 0
    while r < nrows:
        n = min(gmax, nrows - r)
        out.append((base + r, n))
        r += n
    return out


def _subs(nrows, gmax=4):
    return _row_groups(nrows, gmax)


def build_program(sim=False, dbg=None):
    nc = bacc.Bacc("TRN2", target_bir_lowering=False, debug=False,
                   enable_asserts=False, num_devices=NCORES)
    d = {}
    def din(name, shape, dt):
        d[name] = nc.dram_tensor(name, shape, dt, kind="ExternalInput").ap()
    din("xr", [128, 72, 130], BF16)
    din("xr8", [128, 72, 130], F8)
    din("xo8", [128, 72, 130], F8)
    din("wc", [128, 3 * NP3 * 256], F8)
    din("cb", [128, 3], F32)
    din("cm", [128, 3], F32)
    din("dwW", [128, 3 * NP5 * 256], F8)
    din("dw3W", [128, 6 * NP3 * 256], F8)
    din("brd", [128, 96], BF16)
    din("pjw", [128, 3 * 128], BF16)
    din("pjb", [128, 1], F32)
    din("m1w", [128, 6 * 128], BF16)
    din("m1b", [128, 6], F32)
    din("m2b", [128, 6], F32)
    din("m3w8", [128, 3 * 256], F8)
    din("m3b", [128, 1], F32)
    din("idt", [128, 128], BF16)
    d["out"] = nc.dram_tensor("out", [128, 64, 128], F32,
                              kind="ExternalOutput").ap()
    if dbg:
        d["dbg"] = nc.dram_tensor("dbg", [128, 9240], BF16,
                                  kind="ExternalOutput").ap()
    with tile.TileContext(nc) as tc:
        _emit(nc, tc, d, sim=sim, dbg=dbg)
    nc.compile()
    return nc


def _emit(nc, tc, d, sim=False, dbg=None):
    import contextlib
    # pin the ACT table to the one set holding Ln+Exp+Copy+Identity so the
    # per-chunk exp/ln reciprocal never triggers table reloads (1.28us each)
    from concourse.hw_specs import get_activation_tables
    tabs = list(get_activation_tables(nc.m.arch).items())
    idx = next(i for i, (_nm, s) in enumerate(tabs)
               if AF.Ln in s and AF.Exp in s and AF.Copy in s
               and AF.Identity in s)
    ld = mybir.InstLoadActFuncSet(name=nc.get_next_instruction_name(),
                                  ins=[], outs=[], act_func_set_id=idx)
    ld.engine = mybir.EngineType.Activation
    nc.scalar.add_instruction(ld)
    ctx = contextlib.ExitStack()
    with ctx:
        wp = ctx.enter_context(tc.tile_pool(name="wp", bufs=1))
        dram = ctx.enter_context(tc.tile_pool(name="dram", bufs=1, space="DRAM"))
        afp = ctx.enter_context(tc.tile_pool(name="afp", bufs=1))

        def wtile(name, shape, dt):
            t = wp.tile(shape, dt, tag=name, name=name)
            nc.sync.dma_start(out=t, in_=d[name])
            return t

        cb = wtile("cb", [128, 3], F32)
        cm = wtile("cm", [128, 3], F32)
        brd = wtile("brd", [128, 96], BF16)
        pjw = wtile("pjw", [128, 3 * 128], BF16)
        pjb = wtile("pjb", [128, 1], F32)
        m1w = wtile("m1w", [128, 6 * 128], BF16)
        m1b = wtile("m1b", [128, 6], F32)
        m2b = wtile("m2b", [128, 6], F32)
        m3b = wtile("m3b", [128, 1], F32)
        ident = wtile("idt", [128, 128], BF16)
        # v-gather staging with a built-in ones column (ks = kv @ ones)
        vcs = []
        for i in range(2):
            vct = wp.tile([128, 2, 16, 9], BF16, tag=f"vc{i}", name=f"vc{i}")
            nc.vector.memset(vct[:, :, :, 8:9], 1.0)
            vcs.append(vct)
        comp = wp.tile([128, 288], BF16, tag="comp", name="comp")
        # att lhsT tiles (bf16); the bf16 AllReduce result scatters into
        # them directly
        lhsT_att = {}
        for a in ATT_SRCS:
            for S in ATT_SRCS[a]:
                bt = wp.tile([128, 108], BF16, tag=f"lat{a}_{S}", name=f"lat{a}_{S}")
                nc.vector.memset(bt, 0.0)
                lhsT_att[(a, S)] = bt

        cc_in = dram.tile([128, 288], BF16, tag="cc_in", name="cc_in")
        cc_out = dram.tile([128, 288], BF16, tag="cc_out", name="cc_out")
        cc_win = dram.tile([128, 4], F32, tag="cc_win", name="cc_win")
        cc_wout = dram.tile([128, 4], F32, tag="cc_wout", name="cc_wout")
        # warm-up collective: absorbs the one-time CC startup latency while
        # P1 computes
        nc.vector.memset(warm := wp.tile([128, 4], F32, tag="warm",
                                         name="warm"), 0.0)
        nc.sync.dma_start(out=cc_win[:], in_=warm)
        if not sim:
            nc.gpsimd.collective_compute(
                "AllReduce", Alu.add,
                replica_groups=[[0, 1], [2, 3], [4, 5], [6, 7]],
                ins=[cc_win.opt()], outs=[cc_wout.opt()])

        attf = afp.tile([128, 66, 128], BF16, tag="attf", name="attf")
        # xr is the ref-residual source (rows lr-1..64 = idx 3..69); it is
        # not needed until P6/7 so its DMA goes last (below, after the conv
        # inputs)
        xr = afp.tile([128, 72, 130], BF16, tag="xr", name="xr")

        with tc.tile_pool(name="qp", bufs=1) as qp, \
             tc.tile_pool(name="msp", bufs=1) as msp:
            dwstack = contextlib.ExitStack()
            dwp = dwstack.enter_context(tc.tile_pool(name="dwp", bufs=1))
            dwW = dwp.tile([128, 3 * NP5 * 256], F8, tag="dwW", name="dwW")
            nc.sync.dma_start(out=dwW[:, 0:4992], in_=d["dwW"][:, 0:4992])
            nc.scalar.dma_start(out=dwW[:, 4992:], in_=d["dwW"][:, 4992:])

            # ---- P1: qkv convs (bf16 out + fp8 copy for dw5) ----
            qkv = []
            qkv8 = []
            with tc.tile_pool(name="xp", bufs=1) as xp, \
                 tc.tile_pool(name="cps", bufs=2, space="PSUM") as cps:
                wc = xp.tile([128, 3 * NP3 * 256], F8, tag="wc", name="wc")
                nc.sync.dma_start(out=wc, in_=d["wc"])
                xr8 = xp.tile([128, 72, 130], F8, tag="xr8", name="xr8")
                xo8 = xp.tile([128, 72, 130], F8, tag="xo8", name="xo8")
                for (a0, a1) in ((0, 24), (24, 48), (48, 72)):
                    nc.sync.dma_start(out=xr8[:, a0:a1, :],
                                      in_=d["xr8"][:, a0:a1, :])
                    nc.scalar.dma_start(out=xo8[:, a0:a1, :],
                                        in_=d["xo8"][:, a0:a1, :])
                nc.sync.dma_start(out=xr[:, 0:36, :], in_=d["xr"][:, 0:36, :])
                nc.scalar.dma_start(out=xr[:, 36:72, :],
                                    in_=d["xr"][:, 36:72, :])
                for j in range(3):
                    # bf16 tile: rows = lr -1..64 (66) + 2 zero pad rows so the
                    # att chunks are uniformly 4 rows wide, cols = x 0..128
                    # (no halo; only the fp8 copy needs conv halos for dw5)
                    qt = qp.tile([128, 66, 128], BF16, tag=f"q{j}", name=f"q{j}")
                    nc.vector.memset(qt[:, 0:1, :], 0.0)
                    qt8 = dwp.tile([128, 70, 132], F8, tag=f"q8{j}",
                                   name=f"q8{j}")
                    nc.vector.memset(qt8[:, 0:3, :], 0.0)
                    nc.vector.memset(qt8[:, 3:70, 0:2], 0.0)
                    nc.vector.memset(qt8[:, 3:70, 130:132], 0.0)
                    src = xr8 if j == 0 else xo8
                    for (r0, nr) in _row_groups(67, base=3):
                        ps = cps.tile([128, 16 * 128], F32, tag="cps", name="cps")
                        psv = ps[:, 0:nr * 128].rearrange("p (r w) -> p r w", w=128)
                        for (sr, sn) in _subs(nr):
                            for p_, (dyA, dxA, dyB, dxB, _ra) in enumerate(
                                    PAIRS3):
                                dlt = (dyB - dyA) * 130 + (dxB - dxA)
                                win = src[:, r0 + sr + dyA:
                                          r0 + sr + dyA + sn, dxA:dxA + 128]
                                rhs = bass.AP(
                                    tensor=win.tensor, offset=win.offset,
                                    ap=[list(win.ap[0]), [dlt, 2],
                                        list(win.ap[1]), list(win.ap[2])])
                                w8 = wc[:, (j * NP3 + p_) * 256:
                                        (j * NP3 + p_) * 256 + 256].rearrange(
                                    "p (k m) -> p k m", k=2)
                                nc.tensor.matmul(
                                    psv[:, sr:sr + sn, :], w8, rhs,
                                    start=(p_ == 0), stop=(p_ == NP3 - 1),
                                    perf_mode=DR)
                        cnt = min(nr, 68 - r0)   # qt rows = old idx 2..67
                        nc.scalar.activation(
                            out=qt[:, r0 - 2:r0 - 2 + cnt, :],
                            in_=psv[:, 0:cnt, :],
                            func=AF.Identity, bias=cb[:, j:j + 1],
                            scale=1.0 / SC)
                        with nc.allow_low_precision(reason="fp8 dw5 input"):
                            nc.vector.tensor_scalar(
                                out=qt8[:, r0:r0 + nr, 2:130],
                                in0=psv[:, 0:nr, :],
                                scalar1=1.0 / SC, scalar2=cb[:, j:j + 1],
                                op0=Alu.mult, op1=Alu.add)
                    qkv.append(qt)
                    qkv8.append(qt8)
            # relu-mask bf16 qkv right away (dw5 uses the raw fp8 copy)
            for t in range(3):
                nc.vector.tensor_scalar_max(out=qkv[t], in0=qkv[t],
                                            scalar1=cm[:, t:t + 1])

            # ---- P2+P4 fused: dw5(fp8 DoubleRow) chunks interleaved with
            #      per-row transposes + kv matmuls ----
            ms = [msp.tile([128, 66, 128], BF16, tag=f"ms{t}", name=f"ms{t}")
                  for t in range(3)]
            with tc.tile_pool(name="dps", bufs=2, space="PSUM") as dps, \
                 tc.tile_pool(name="kps", bufs=1, space="PSUM") as kps, \
                 tc.tile_pool(name="tpp", bufs=4, space="PSUM") as tpp, \
                 tc.tile_pool(name="mtp", bufs=8) as mtp:
                kvT = [kps.tile([128, 144], F32, tag=f"kvT{h}", name=f"kvT{h}")
                       for h in range(2)]

                def kv_row(r):
                    # all 6 row transposes on PE (identity transpose-matmul);
                    # psum->sbuf copies alternate DVE/ACT
                    mT = mtp.tile([128, 768], BF16, tag="mT", name="mT")
                    for i, (src, dst0) in enumerate(
                            [(qkv[0][:, 1 + r, :], 0),
                             (ms[0][:, 1 + r, :], 384),
                             (qkv[1][:, 1 + r, :], 128),
                             (ms[1][:, 1 + r, :], 512),
                             (qkv[2][:, 1 + r, :], 256),
                             (ms[2][:, 1 + r, :], 640)]):
                        tp = tpp.tile([128, 128], BF16, tag="tp", name="tp")
                        nc.tensor.transpose(tp, src, ident)
                        if (i + r) % 2 == 0:
                            nc.vector.tensor_copy(
                                out=mT[:, dst0:dst0 + 128], in_=tp)
                        else:
                            nc.scalar.activation(
                                out=mT[:, dst0:dst0 + 128], in_=tp,
                                func=AF.Copy)
                    mg = mT.rearrange("p (g c) -> p g c", c=24)
                    kc = mtp.tile([128, 256], BF16, tag="kc", name="kc")
                    nc.vector.tensor_copy(
                        out=kc.rearrange("p (g e) -> p g e", e=8),
                        in_=mg[:, :, 8:16])
                    vc = vcs[r % 2]
                    nc.vector.tensor_copy(
                        out=vc.rearrange("p h g e -> p (h g) e")[:, :, 0:8],
                        in_=mg[:, :, 16:24])
                    for h in range(2):
                        kcols = kc[:, 128 * h:128 * h + 128]
                        nc.tensor.matmul(kvT[h], kcols, vc[:, h, :, :],
                                         start=(r == 0), stop=(r == 63))

                rdone = 0
                for ci, (c0, cn) in enumerate(_subs(66)):
                    for t in range(3):
                        ps = dps.tile([128, 512], F32, tag="dps", name="dps")
                        q8 = qkv8[t]
                        for p_, (dyA, dxA, dyB, dxB, _ra) in enumerate(
                                PAIRS5):
                            dlt = (dyB - dyA) * 132 + (dxB - dxA)
                            win = q8[:, c0 + dyA:c0 + dyA + cn,
                                     dxA:dxA + 128]
                            rhs = bass.AP(
                                tensor=win.tensor, offset=win.offset,
                                ap=[list(win.ap[0]), [dlt, 2],
                                    list(win.ap[1]), list(win.ap[2])])
                            w8 = dwW[:, (t * NP5 + p_) * 256:
                                     (t * NP5 + p_) * 256 + 256].rearrange(
                                "p (k m) -> p k m", k=2)
                            nc.tensor.matmul(ps[:, 0:cn * 128], w8, rhs,
                                             start=(p_ == 0),
                                             stop=(p_ == NP5 - 1),
                                             perf_mode=DR)
                        nc.scalar.activation(
                            out=ms[t][:, c0:c0 + cn, :],
                            in_=ps[:, 0:cn * 128].rearrange(
                                "p (r w) -> p r w", w=128),
                            func=AF.Copy, scale=1.0 / S5)
                        nc.vector.tensor_scalar_max(
                            out=ms[t][:, c0:c0 + cn, :],
                            in0=ms[t][:, c0:c0 + cn, :],
                            scalar1=cm[:, t:t + 1])
                    lim = min(64, c0 + cn - 1)
                    while rdone < lim:
                        kv_row(rdone)
                        rdone += 1
                while rdone < 64:
                    kv_row(rdone)
                    rdone += 1
                with nc.allow_low_precision(reason="bf16 kv exchange"):
                    for h in range(2):
                        nc.vector.tensor_copy(
                            out=comp[:, 144 * h:144 * h + 144], in_=kvT[h])
            dwstack.close()
            if dbg and dbg.startswith("ms"):
                ti = int(dbg[2])
                nc.sync.dma_start(
                    out=d["dbg"][:, 0:66 * 128].rearrange(
                        "p (r w) -> p r w", w=128), in_=ms[ti])
            if dbg and dbg.startswith("qm"):
                ti = int(dbg[2])
                nc.sync.dma_start(
                    out=d["dbg"][:, 0:66 * 128].rearrange(
                        "p (r w) -> p r w", w=128), in_=qkv[ti])

            # ---- P5: AllReduce + scatter ----
            nc.sync.dma_start(out=cc_in[:], in_=comp)
            if sim:
                nc.sync.dma_start(out=cc_out[:], in_=cc_in[:])
            else:
                nc.gpsimd.collective_compute(
                    "AllReduce", Alu.add,
                    replica_groups=[[0, 1], [2, 3], [4, 5], [6, 7]],
                    ins=[cc_in.opt()], outs=[cc_out.opt()])
            for g in range(32):
                a, gl9 = g // 12, g % 12
                h, gl = g // 16, g % 16
                S, row0 = q_chan(g, 0) // 128, q_chan(g, 0) % 128
                L = lhsT_att[(a, S)].rearrange("p (dd gl) -> p dd gl", gl=12)
                eng = (nc.sync, nc.scalar, nc.gpsimd)[g % 3]
                eng.dma_start(
                    out=L[row0:row0 + 8, 0:9, gl9:gl9 + 1],
                    in_=cc_out[8 * gl:8 * gl + 8,
                               144 * h + 9 * gl:144 * h + 9 * gl + 9])
            # ---- P6+P7 fused: att9 + division + proj + residual per chunk ----
            # denominator reciprocal on ACT via exp(-ln(x + eps)): the exp/ln
            # table set also holds copy/identity/relu so no table reloads; the
            # DVE 8-pass iterative reciprocal is gone entirely.
            with tc.tile_pool(name="aps", bufs=2, space="PSUM") as aps, \
                 tc.tile_pool(name="bps", bufs=1, space="PSUM") as bpsp, \
                 tc.tile_pool(name="jps", bufs=1, space="PSUM") as jpsp, \
                 tc.tile_pool(name="dnp", bufs=3) as dnp:
                epsb = dnp.tile([128, 1], F32, tag="epsb", name="epsb",
                                bufs=1)
                nc.vector.memset(epsb, 1e-15)

                def att_rhs(S, c0, cn):
                    if S < 3:
                        return qkv[S][:, c0:c0 + cn, :]
                    return ms[S - 3][:, c0:c0 + cn, :]

                for ci, (c0, cn) in enumerate(_subs(66)):
                    cw = cn * 128
                    psl = aps.tile([108, 3 * 512], F32, tag="aps", name="aps")
                    for a in range(3):
                        srcs = ATT_SRCS[a]
                        for i, S in enumerate(srcs):
                            nc.tensor.matmul(psl[:, a * 512:a * 512 + cw],
                                             lhsT_att[(a, S)],
                                             att_rhs(S, c0, cn), start=(i == 0),
                                             stop=(i == len(srcs) - 1))
                    lnden = dnp.tile([12, 3 * 512], F32, tag="lnden",
                                     name="lnden")
                    nc.scalar.activation(out=lnden, in_=psl[96:108, :],
                                         func=AF.Ln, bias=epsb[0:12, 0:1],
                                         scale=1.0)
                    rden = dnp.tile([12, 3 * 512], BF16, tag="rden",
                                    name="rden")
                    with nc.allow_low_precision(reason="den recip to bf16"):
                        nc.scalar.activation(out=rden, in_=lnden, func=AF.Exp,
                                             scale=-1.0)
                    jp = jpsp.tile([128, 512], F32, tag="jps", name="jps")
                    for a in range(3):
                        bp = bpsp.tile([96, 512], F32, tag="bps", name="bps")
                        nc.tensor.matmul(bp, brd[0:12, :],
                                         rden[:, a * 512:a * 512 + 512],
                                         start=True, stop=True)
                        recb = dnp.tile([96, 512], BF16, tag="recb",
                                        name="recb")
                        with nc.allow_low_precision(reason="recb bf16"):
                            if a == 1:
                                nc.scalar.activation(out=recb, in_=bp,
                                                     func=AF.Copy)
                            else:
                                nc.vector.tensor_copy(out=recb, in_=bp)
                        attc = dnp.tile([96, 512], BF16, tag="attc",
                                        name="attc")
                        with nc.allow_low_precision(reason="att to bf16"):
                            nc.vector.tensor_mul(
                                out=attc, in0=psl[0:96, a * 512:a * 512 + 512],
                                in1=recb)
                        nc.tensor.matmul(jp,
                                         pjw[0:96, a * 128:a * 128 + 128],
                                         attc,
                                         start=(a == 0), stop=(a == 2))
                    aB = dnp.tile([128, 512], BF16, tag="aB", name="aB")
                    nc.scalar.activation(out=aB, in_=jp,
                                         func=AF.Identity, bias=pjb[:, 0:1],
                                         scale=1.0)
                    nc.vector.tensor_add(
                        out=attf[:, c0:c0 + cn, :],
                        in0=aB[:, 0:cw].rearrange("p (r w) -> p r w", w=128),
                        in1=xr[:, c0 + 3:c0 + 3 + cn, 1:129])

        if dbg == "attf":
            nc.sync.dma_start(
                out=d["dbg"][:, 0:66 * 128].rearrange(
                    "p (r w) -> p r w", w=128), in_=attf)
        # ---- P8: mb1 + hswish -> h1 fp8 (SBUF resident) ----
        with tc.tile_pool(name="h1p", bufs=1) as h1p, \
             tc.tile_pool(name="lwp", bufs=1) as lwp:
            dw3W = lwp.tile([128, 6 * NP3 * 256], F8, tag="dw3W", name="dw3W")
            nc.sync.dma_start(out=dw3W, in_=d["dw3W"])
            h1 = []
            for t in range(6):
                ht = h1p.tile([128, 66, 130], F8, tag=f"h1{t}", name=f"h1{t}")
                nc.vector.memset(ht[:, 0:1, :], 0.0)
                nc.vector.memset(ht[:, 1:66, 0:1], 0.0)
                nc.vector.memset(ht[:, 1:66, 129:130], 0.0)
                h1.append(ht)
            with tc.tile_pool(name="m1ps", bufs=2, space="PSUM") as m1ps, \
                 tc.tile_pool(name="hwp", bufs=2) as hwp:
                for (r0, nr) in _row_groups(65, base=1):
                    for t in range(6):
                        ps = m1ps.tile([128, 16 * 128], F32, tag="m1ps", name="m1ps")
                        for (sr, sn) in _subs(nr):
                            nc.tensor.matmul(
                                ps[:, sr * 128:(sr + sn) * 128],
                                m1w[:, t * 128:t * 128 + 128],
                                attf[:, r0 + sr:r0 + sr + sn, :],
                                start=True, stop=True)
                        pw_ = ps[:, 0:nr * 128]
                        # 6*hswish(x) = min(relu(x+3),6)*x in 2 DVE ops; the
                        # 1/6 is folded into the host dw3 weights
                        th = hwp.tile([128, 16 * 128], BF16, tag="th", name="th")
                        xh = hwp.tile([128, 16 * 128], BF16, tag="xh", name="xh")
                        nc.scalar.activation(out=xh[:, 0:nr * 128], in_=pw_,
                                             func=AF.Identity,
                                             bias=m1b[:, t:t + 1], scale=1.0)
                        nc.vector.tensor_scalar(
                            out=th[:, 0:nr * 128], in0=xh[:, 0:nr * 128],
                            scalar1=3.0, scalar2=0.0,
                            op0=Alu.add, op1=Alu.max)
                        if t % 2 == 0:
                            with nc.allow_low_precision(reason="h1 fp8"):
                                nc.vector.scalar_tensor_tensor(
                                    out=h1[t][:, r0:r0 + nr, 1:129],
                                    in0=th[:, 0:nr * 128].rearrange(
                                        "p (r w) -> p r w", w=128),
                                    scalar=6.0,
                                    in1=xh[:, 0:nr * 128].rearrange(
                                        "p (r w) -> p r w", w=128),
                                    op0=Alu.min, op1=Alu.mult)
                        else:
                            hb = hwp.tile([128, 16 * 128], BF16, tag="hb",
                                          name="hb")
                            nc.vector.scalar_tensor_tensor(
                                out=hb[:, 0:nr * 128], in0=th[:, 0:nr * 128],
                                scalar=6.0, in1=xh[:, 0:nr * 128],
                                op0=Alu.min, op1=Alu.mult)
                            with nc.allow_low_precision(reason="h1 fp8"):
                                nc.scalar.activation(
                                    out=h1[t][:, r0:r0 + nr, 1:129],
                                    in_=hb[:, 0:nr * 128].rearrange(
                                        "p (r w) -> p r w", w=128),
                                    func=AF.Copy)

            # ---- P9: dw3 (fp8 DoubleRow) + hswish + mb3 + final adds ----
            with tc.tile_pool(name="d3ps", bufs=2, space="PSUM") as d3ps, \
                 tc.tile_pool(name="m3ps", bufs=2, space="PSUM") as m3ps, \
                 tc.tile_pool(name="h2p", bufs=2) as h2p, \
                 tc.tile_pool(name="osp", bufs=2) as osp:
                m3w8 = lwp.tile([128, 3 * 256], F8, tag="m3w8", name="m3w8")
                nc.scalar.dma_start(out=m3w8, in_=d["m3w8"])
                for q in range(8):   # 8-row half-bands, out rows lr 8q..8q+8
                    ps = m3ps.tile([128, 8 * 128], F32, tag="m3ps", name="m3ps")
                    h2s = h2p.tile([128, 6, 8 * 128], F8, tag="h2s",
                                   name="h2s")
                    for t in range(6):
                        dp = d3ps.tile([128, 8 * 128], F32, tag="d3ps", name="d3ps")
                        hh = h1[t]
                        for (sr, sn) in _subs(8):
                            for p_, (dyA, dxA, dyB, dxB, _ra) in enumerate(
                                    PAIRS3):
                                dlt = (dyB - dyA) * 130 + (dxB - dxA)
                                win = hh[:, 8 * q + sr + dyA:
                                         8 * q + sr + dyA + sn,
                                         dxA:dxA + 128]
                                rhs = bass.AP(
                                    tensor=win.tensor, offset=win.offset,
                                    ap=[list(win.ap[0]), [dlt, 2],
                                        list(win.ap[1]), list(win.ap[2])])
                                w8 = dw3W[:, (t * NP3 + p_) * 256:
                                          (t * NP3 + p_) * 256 + 256].rearrange(
                                    "p (k m) -> p k m", k=2)
                                nc.tensor.matmul(
                                    dp[:, sr * 128:(sr + sn) * 128], w8, rhs,
                                    start=(p_ == 0), stop=(p_ == NP3 - 1),
                                    perf_mode=DR)
                        th = h2p.tile([128, 8 * 128], BF16, tag="th2", name="th2")
                        xh = h2p.tile([128, 8 * 128], BF16, tag="xh2", name="xh2")
                        nc.scalar.activation(out=xh, in_=dp, func=AF.Identity,
                                             bias=m2b[:, t:t + 1], scale=1.0 / S3)
                        nc.vector.tensor_scalar(
                            out=th, in0=xh, scalar1=3.0, scalar2=0.0,
                            op0=Alu.add, op1=Alu.max)
                        with nc.allow_low_precision(reason="h2 fp8"):
                            nc.vector.scalar_tensor_tensor(
                                out=h2s[:, t, :], in0=th, scalar=6.0, in1=xh,
                                op0=Alu.min, op1=Alu.mult)
                    for pr in range(3):
                        for (sr, sn) in _subs(8):
                            win = h2s[:, 2 * pr, sr * 128:(sr + sn) * 128]
                            rhs = bass.AP(
                                tensor=win.tensor, offset=win.offset,
                                ap=[list(win.ap[0]), [1024, 2],
                                    list(win.ap[1])])
                            w8 = m3w8[:, pr * 256:pr * 256 + 256].rearrange(
                                "p (k m) -> p k m", k=2)
                            nc.tensor.matmul(
                                ps[:, sr * 128:(sr + sn) * 128], w8, rhs,
                                start=(pr == 0), stop=(pr == 2),
                                perf_mode=DR)
                    o1 = osp.tile([128, 8 * 128], F32, tag="o1", name="o1")
                    nc.scalar.activation(out=o1, in_=ps, func=AF.Identity,
                                         bias=m3b[:, 0:1], scale=1.0 / S3M)
                    nc.vector.tensor_add(
                        out=o1, in0=o1,
                        in1=attf[:, 8 * q + 1:8 * q + 9, :].rearrange(
                            "p r w -> p (r w)"))
                    nc.sync.dma_start(out=d["out"][:, 8 * q:8 * q + 8, :],
                                      in_=o1.rearrange("p (r w) -> p r w", w=128))


# ====================== host side ======================

def _prep_shared(inp):
    f32 = np.float32
    out = {}
    pw = inp["agg_pw_w"][:, :, 0, 0]          # [384, 8]
    w5 = inp["agg_dw_w"][:, 0, :, :]          # [384, 5, 5]
    w3 = inp["mb2_w"][:, 0, :, :]             # [768, 3, 3]
    for s in (0, 1):
        w = {}
        wc = np.zeros((128, 3 * NP3 * 256), f32)
        for j, cw in enumerate((inp["wq"], inp["wk"], inp["wv"])):
            def c_mat(dy, dx):
                dyy = 2 - dy if s == 1 else dy
                return cw[:, :, dyy, dx].T * SC
            for p_, (dyA, dxA, dyB, dxB, realA) in enumerate(PAIRS3):
                k = (j * NP3 + p_) * 256
                if realA:
                    wc[:, k:k + 128] = c_mat(dyA, dxA)
                wc[:, k + 128:k + 256] = c_mat(dyB, dxB)
        w["wc"] = wc.astype(F8NP)
        w["cb"] = np.stack([inp["bq"], inp["bk"], inp["bv"]], 1).astype(f32)
        m = np.arange(384)
        w["cm"] = np.where((m % 24) < 16, 0.0, -1e9).astype(f32).reshape(3, 128).T.copy()
        # fused dw5x5 + grouped pw block-diag weights, fp8 DoubleRow pairs
        def dw5_mat(t, dy, dx):
            dyy = 4 - dy if s == 1 else dy
            M = np.zeros((128, 128), f32)
            for b in range(16):
                i0 = 8 * b
                blk = (w5[128 * t + i0:128 * t + i0 + 8, dyy, dx][:, None]
                       * pw[128 * t + i0:128 * t + i0 + 8, :].T)
                M[i0:i0 + 8, i0:i0 + 8] = blk
            return M * S5
        dwW = np.zeros((128, 3 * NP5 * 256), f32)
        for t in range(3):
            for p_, (dyA, dxA, dyB, dxB, realA) in enumerate(PAIRS5):
                k = (t * NP5 + p_) * 256
                if realA:
                    dwW[:, k:k + 128] = dw5_mat(t, dyA, dxA)
                dwW[:, k + 128:k + 256] = dw5_mat(t, dyB, dxB)
        w["dwW"] = dwW.astype(F8NP)
        # dw3 diagonal weights, fp8 DoubleRow vertical pairs
        def dw3_mat(t, dy, dx):
            dyy = 2 - dy if s == 1 else dy
            M = np.zeros((128, 128), f32)
            M[np.arange(128), np.arange(128)] = \
                w3[128 * t:128 * t + 128, dyy, dx] * (S3 / 6.0)
            return M
        dw3W = np.zeros((128, 6 * NP3 * 256), f32)
        for t in range(6):
            for p_, (dyA, dxA, dyB, dxB, realA) in enumerate(PAIRS3):
                k = (t * NP3 + p_) * 256
                if realA:
                    dw3W[:, k:k + 128] = dw3_mat(t, dyA, dxA)
                dw3W[:, k + 128:k + 256] = dw3_mat(t, dyB, dxB)
        w["dw3W"] = dw3W.astype(F8NP)
        # one-hot broadcast for denominators: rows 96+gl9 -> out col o (gl9=o%12)
        brd = np.zeros((128, 96), f32)
        o = np.arange(96)
        brd[o % 12, o] = 1.0
        w["brd"] = brd.astype(BF)
        s1 = inp["bn1_g"] / np.sqrt(inp["bn1_v"] + BN_EPS)
        b1 = inp["bn1_b"] - inp["bn1_m"] * s1
        Wp = inp["attn_proj_w"][:, :, 0, 0] * s1[:, None]
        pjw = np.zeros((128, 3 * 128), f32)
        for g in range(32):
            a, gl9 = g // 12, g % 12
            for dd in range(8):
                pjw[12 * dd + gl9, a * 128:a * 128 + 128] = Wp[:, 8 * g + dd]
        w["pjw"] = pjw.astype(BF)
        w["pjb"] = b1.reshape(128, 1).astype(f32)
        m1w = np.zeros((128, 6 * 128), f32)
        for t in range(6):
            m1w[:, t * 128:t * 128 + 128] = inp["mb1_w"][128 * t:128 * t + 128, :, 0, 0].T
        w["m1w"] = m1w.astype(BF)
        w["m1b"] = inp["mb1_b"].reshape(6, 128).T.copy().astype(f32)
        w["m2b"] = inp["mb2_b"].reshape(6, 128).T.copy().astype(f32)
        s2 = inp["bn2_g"] / np.sqrt(inp["bn2_v"] + BN_EPS)
        b2 = inp["bn2_b"] - inp["bn2_m"] * s2
        W3 = inp["mb3_w"][:, :, 0, 0] * s2[:, None]
        m3w8 = np.zeros((128, 3 * 256), f32)
        for t in range(6):
            half = 128 * (t % 2)
            k = (t // 2) * 256 + half
            m3w8[:, k:k + 128] = \
                W3[:, 128 * t:128 * t + 128].T * (S3M / 6.0)
        w["m3w8"] = m3w8.astype(F8NP)
        w["m3b"] = b2.reshape(128, 1).astype(f32)
        w["idt"] = np.eye(128, dtype=f32).astype(BF)
        out[s] = w
    return out


def _prep_core(inp, b, s):
    f32 = np.float32
    ref = inp["ref_features"][b]
    oth = inp["other_features"][b]
    if s == 1:
        ref = ref[:, ::-1, :]
        oth = oth[:, ::-1, :]
    xr = np.zeros((128, 72, 130), f32)
    xo = np.zeros((128, 72, 130), f32)
    xr[:, 4:72, 1:129] = ref[:, 0:68, :]
    xo[:, 4:72, 1:129] = oth[:, 0:68, :]
    return {"xr": xr.astype(BF), "xr8": xr.astype(F8NP),
            "xo8": xo.astype(F8NP)}


def make_in_maps(inp):
    ws = _prep_shared(inp)
    in_maps = []
    for c in range(NCORES):
        b, s = c // 2, c % 2
        m = dict(ws[s])
        m.update(_prep_core(inp, b, s))
        in_maps.append(m)
    return in_maps


def kernel(**inputs):
    inp = {k: np.asarray(v) for k, v in inputs.items()}
    if "nc" not in _CACHE:
        _CACHE["nc"] = build_program()
    nc = _CACHE["nc"]
    in_maps = make_in_maps(inp)
    res = bass_utils.run_bass_kernel_spmd(nc, in_maps,
                                          core_ids=list(range(NCORES)))
    out = np.zeros((4, 128, 128, 128), np.float32)
    for c in range(NCORES):
        b, s = c // 2, c % 2
        o = res.results[c]["out"]
        if s == 1:
            o = o[:, ::-1, :]
        out[b, :, 64 * s:64 * s + 64, :] = o
    return out



# revision 34
# speedup vs baseline: 4.2909x; 4.1807x over previous
"""EfficientViT attention block on 8 TRN2 NeuronCores.

Sharding: 8 cores = 4 images x 2 row-halves (64 rows each + halos).
s=1 cores receive a vertically flipped image + dy-flipped conv weights so the
SPMD program is identical on all cores. The linear-attention kv matrices are
partial sums over each core's own 64 rows and are combined with a pairwise
bf16 AllReduce ([128,288]; ks rides along as a ones column of v).

Key structure (all intermediates SBUF-resident):
  P1  qkv 3x3 convs as fp8 DoubleRow pairs (5 passes/conv) -> qt[3] bf16
      + qt8[3] fp8 (dw5 input with halos)
  P2  dw5x5 fused with grouped pw -> fp8 DR block-diag matmuls, 13
      passes/tile (pairs may span dy AND dx; any ifmap delta except 1 works)
  P4  per-row PE transposes + kv PSUM accumulation [128,144] per half
      (8 kv dims + 1 ks per group, interleaved)
  P5  warm-up collective at t=0 absorbs CC startup; bf16 AllReduce of
      [128,288]; 32 contiguous 9-col DMAs scatter straight into the bf16
      att lhsT tiles over 3 DMA queues
  P6/7 per 4-row chunk: att matmuls -> denominator reciprocal on ACT as
      exp(-ln(x+eps)) (one pinned act table, no reloads) -> PE one-hot
      broadcast -> DVE mul -> proj matmul -> +ref -> attf
  P8  mb1 1x1 (PE) + hswish in 2 DVE ops (6*hswish = min(relu(x+3),6)*x,
      the /6 folded into the dw3 weights) -> h1[6] fp8
  P9  dw3 fp8 DR diag matmuls (5 passes) + hswish + mb3 as fp8 DR pairs
      (3 passes) + residual
"""
import numpy as np
import ml_dtypes

import concourse.bass as bass
import concourse.bacc as bacc
import concourse.tile as tile
from concourse import mybir
from concourse import bass_utils

F32 = mybir.dt.float32
BF16 = mybir.dt.bfloat16
F8 = mybir.dt.float8e4
Alu = mybir.AluOpType
AF = mybir.ActivationFunctionType
DR = mybir.MatmulPerfMode.DoubleRow
BF = ml_dtypes.bfloat16
F8NP = mybir.dt.np(mybir.dt.float8e4)
S5 = 256.0   # fp8 scale for fused dw5+pw weights
S3 = 384.0   # fp8 scale for dw3 weights (on 6*hswish h1)
S3M = 512.0  # fp8 scale for mb3 weights (on 6*hswish h2)

BN_EPS = 1e-5
NCORES = 8

_CACHE = {}

ATT_SRCS = {0: (0, 1, 2), 1: (2, 3, 4), 2: (4, 5)}  # att9 tile a -> multi tiles

# DoubleRow tap pairings: (dyA, dxA, dyB, dxB, realA). The ifmap AP delta is
# (dyB-dyA)*row_pitch + (dxB-dxA); HW accepts any delta except 1. Pads
# (realA=False) put a zero weight in the A slot.
PAIRS5 = ([(0, dx, 1, dx, True) for dx in range(5)]
          + [(2, dx, 3, dx, True) for dx in range(5)]
          + [(4, 0, 4, 2, True), (4, 1, 4, 3, True), (4, 2, 4, 4, False)])
PAIRS3 = ([(0, dx, 1, dx, True) for dx in range(3)]
          + [(2, 0, 2, 2, True), (0, 1, 2, 1, False)])
NP5 = len(PAIRS5)   # 13
NP3 = len(PAIRS3)   # 5
SC = 512.0          # fp8 scale for qkv conv weights


def q_chan(g, e):
    return 24 * g + e if g < 16 else 384 + 24 * (g - 16) + e


def _row_groups(nrows, gmax=16, base=0):
    out, r = [], 0
    while r < nrows:
        n = min(gmax, nrows - r)
        out.append((base + r, n))
        r += n
    return out


def _subs(nrows, gmax=4):
    return _row_groups(nrows, gmax)


def build_program(sim=False, dbg=None):
    nc = bacc.Bacc("TRN2", target_bir_lowering=False, debug=False,
                   enable_asserts=False, num_devices=NCORES)
    d = {}
    def din(name, shape, dt):
        d[name] = nc.dram_tensor(name, shape, dt, kind="ExternalInput").ap()
    din("xr", [128, 72, 130], BF16)
    din("xr8", [128, 72, 130], F8)
    din("xo8", [128, 72, 130], F8)
    din("wc", [128, 3 * NP3 * 256], F8)
    din("cb", [128, 3], F32)
    din("cm", [128, 3], F32)
    din("dwW", [128, 3 * NP5 * 256], F8)
    din("dw3W", [128, 6 * NP3 * 256], F8)
    din("brd", [128, 96], BF16)
    din("pjw", [128, 3 * 128], BF16)
    din("pjb", [128, 1], F32)
    din("m1w", [128, 6 * 128], BF16)
    din("m1b", [128, 6], F32)
    din("m2b", [128, 6], F32)
    din("m3w8", [128, 3 * 256], F8)
    din("m3b", [128, 1], F32)
    din("idt", [128, 128], BF16)
    d["out"] = nc.dram_tensor("out", [128, 64, 128], F32,
                              kind="ExternalOutput").ap()
    if dbg:
        d["dbg"] = nc.dram_tensor("dbg", [128, 9240], BF16,
                                  kind="ExternalOutput").ap()
    with tile.TileContext(nc) as tc:
        _emit(nc, tc, d, sim=sim, dbg=dbg)
    nc.compile()
    return nc


def _emit(nc, tc, d, sim=False, dbg=None):
    import contextlib
    # pin the ACT table to the one set holding Ln+Exp+Copy+Identity so the
    # per-chunk exp/ln reciprocal never triggers table reloads (1.28us each)
    from concourse.hw_specs import get_activation_tables
    tabs = list(get_activation_tables(nc.m.arch).items())
    idx = next(i for i, (_nm, s) in enumerate(tabs)
               if AF.Ln in s and AF.Exp in s and AF.Copy in s
               and AF.Identity in s)
    ld = mybir.InstLoadActFuncSet(name=nc.get_next_instruction_name(),
                                  ins=[], outs=[], act_func_set_id=idx)
    ld.engine = mybir.EngineType.Activation
    nc.scalar.add_instruction(ld)
    ctx = contextlib.ExitStack()
    with ctx:
        wp = ctx.enter_context(tc.tile_pool(name="wp", bufs=1))
        dram = ctx.enter_context(tc.tile_pool(name="dram", bufs=1, space="DRAM"))
        afp = ctx.enter_context(tc.tile_pool(name="afp", bufs=1))

        def wtile(name, shape, dt):
            t = wp.tile(shape, dt, tag=name, name=name)
            nc.sync.dma_start(out=t, in_=d[name])
            return t

        cb = wtile("cb", [128, 3], F32)
        cm = wtile("cm", [128, 3], F32)
        brd = wtile("brd", [128, 96], BF16)
        pjw = wtile("pjw", [128, 3 * 128], BF16)
        pjb = wtile("pjb", [128, 1], F32)
        m1w = wtile("m1w", [128, 6 * 128], BF16)
        m1b = wtile("m1b", [128, 6], F32)
        m2b = wtile("m2b", [128, 6], F32)
        m3b = wtile("m3b", [128, 1], F32)
        ident = wtile("idt", [128, 128], BF16)
        # v-gather staging with a built-in ones column (ks = kv @ ones)
        vcs = []
        for i in range(2):
            vct = wp.tile([128, 2, 16, 9], BF16, tag=f"vc{i}", name=f"vc{i}")
            nc.vector.memset(vct[:, :, :, 8:9], 1.0)
            vcs.append(vct)
        comp = wp.tile([128, 288], BF16, tag="comp", name="comp")
        # att lhsT tiles (bf16); the bf16 AllReduce result scatters into
        # them directly
        lhsT_att = {}
        for a in ATT_SRCS:
            for S in ATT_SRCS[a]:
                bt = wp.tile([128, 108], BF16, tag=f"lat{a}_{S}", name=f"lat{a}_{S}")
                nc.vector.memset(bt, 0.0)
                lhsT_att[(a, S)] = bt

        cc_in = dram.tile([128, 288], BF16, tag="cc_in", name="cc_in")
        cc_out = dram.tile([128, 288], BF16, tag="cc_out", name="cc_out")
        cc_win = dram.tile([128, 4], F32, tag="cc_win", name="cc_win")
        cc_wout = dram.tile([128, 4], F32, tag="cc_wout", name="cc_wout")
        # warm-up collective: absorbs the one-time CC startup latency while
        # P1 computes
        nc.vector.memset(warm := wp.tile([128, 4], F32, tag="warm",
                                         name="warm"), 0.0)
        nc.sync.dma_start(out=cc_win[:], in_=warm)
        if not sim:
            nc.gpsimd.collective_compute(
                "AllReduce", Alu.add,
                replica_groups=[[0, 1], [2, 3], [4, 5], [6, 7]],
                ins=[cc_win.opt()], outs=[cc_wout.opt()])

        attf = afp.tile([128, 66, 128], BF16, tag="attf", name="attf")
        # xr is the ref-residual source (rows lr-1..64 = idx 3..69); it is
        # not needed until P6/7 so its DMA goes last (below, after the conv
        # inputs)
        xr = afp.tile([128, 72, 130], BF16, tag="xr", name="xr")

        with tc.tile_pool(name="qp", bufs=1) as qp, \
             tc.tile_pool(name="msp", bufs=1) as msp:
            dwstack = contextlib.ExitStack()
            dwp = dwstack.enter_context(tc.tile_pool(name="dwp", bufs=1))
            dwW = dwp.tile([128, 3 * NP5 * 256], F8, tag="dwW", name="dwW")
            nc.sync.dma_start(out=dwW[:, 0:4992], in_=d["dwW"][:, 0:4992])
            nc.scalar.dma_start(out=dwW[:, 4992:], in_=d["dwW"][:, 4992:])

            # ---- P1: qkv convs (bf16 out + fp8 copy for dw5) ----
            qkv = []
            qkv8 = []
            with tc.tile_pool(name="xp", bufs=1) as xp, \
                 tc.tile_pool(name="cps", bufs=2, space="PSUM") as cps:
                wc = xp.tile([128, 3 * NP3 * 256], F8, tag="wc", name="wc")
                nc.sync.dma_start(out=wc, in_=d["wc"])
                xr8 = xp.tile([128, 72, 130], F8, tag="xr8", name="xr8")
                xo8 = xp.tile([128, 72, 130], F8, tag="xo8", name="xo8")
                for (a0, a1) in ((0, 24), (24, 48), (48, 72)):
                    nc.sync.dma_start(out=xr8[:, a0:a1, :],
                                      in_=d["xr8"][:, a0:a1, :])
                    nc.scalar.dma_start(out=xo8[:, a0:a1, :],
                                        in_=d["xo8"][:, a0:a1, :])
                nc.sync.dma_start(out=xr[:, 0:36, :], in_=d["xr"][:, 0:36, :])
                nc.scalar.dma_start(out=xr[:, 36:72, :],
                                    in_=d["xr"][:, 36:72, :])
                for j in range(3):
                    # bf16 tile: rows = lr -1..64 (66) + 2 zero pad rows so the
                    # att chunks are uniformly 4 rows wide, cols = x 0..128
                    # (no halo; only the fp8 copy needs conv halos for dw5)
                    qt = qp.tile([128, 66, 128], BF16, tag=f"q{j}", name=f"q{j}")
                    nc.vector.memset(qt[:, 0:1, :], 0.0)
                    qt8 = dwp.tile([128, 70, 132], F8, tag=f"q8{j}",
                                   name=f"q8{j}")
                    nc.vector.memset(qt8[:, 0:3, :], 0.0)
                    nc.vector.memset(qt8[:, 3:70, 0:2], 0.0)
                    nc.vector.memset(qt8[:, 3:70, 130:132], 0.0)
                    src = xr8 if j == 0 else xo8
                    for (r0, nr) in _row_groups(67, base=3):
                        ps = cps.tile([128, 16 * 128], F32, tag="cps", name="cps")
                        psv = ps[:, 0:nr * 128].rearrange("p (r w) -> p r w", w=128)
                        for (sr, sn) in _subs(nr):
                            for p_, (dyA, dxA, dyB, dxB, _ra) in enumerate(
                                    PAIRS3):
                                dlt = (dyB - dyA) * 130 + (dxB - dxA)
                                win = src[:, r0 + sr + dyA:
                                          r0 + sr + dyA + sn, dxA:dxA + 128]
                                rhs = bass.AP(
                                    tensor=win.tensor, offset=win.offset,
                                    ap=[list(win.ap[0]), [dlt, 2],
                                        list(win.ap[1]), list(win.ap[2])])
                                w8 = wc[:, (j * NP3 + p_) * 256:
                                        (j * NP3 + p_) * 256 + 256].rearrange(
                                    "p (k m) -> p k m", k=2)
                                nc.tensor.matmul(
                                    psv[:, sr:sr + sn, :], w8, rhs,
                                    start=(p_ == 0), stop=(p_ == NP3 - 1),
                                    perf_mode=DR)
                        cnt = min(nr, 68 - r0)   # qt rows = old idx 2..67
                        nc.scalar.activation(
                            out=qt[:, r0 - 2:r0 - 2 + cnt, :],
                            in_=psv[:, 0:cnt, :],
                            func=AF.Identity, bias=cb[:, j:j + 1],
                            scale=1.0 / SC)
                        with nc.allow_low_precision(reason="fp8 dw5 input"):
                            nc.vector.tensor_scalar(
                                out=qt8[:, r0:r0 + nr, 2:130],
                                in0=psv[:, 0:nr, :],
                                scalar1=1.0 / SC, scalar2=cb[:, j:j + 1],
                                op0=Alu.mult, op1=Alu.add)
                    qkv.append(qt)
                    qkv8.append(qt8)
            # relu-mask bf16 qkv right away (dw5 uses the raw fp8 copy)
            for t in range(3):
                nc.vector.tensor_scalar_max(out=qkv[t], in0=qkv[t],
                                            scalar1=cm[:, t:t + 1])

            # ---- P2+P4 fused: dw5(fp8 DoubleRow) chunks interleaved with
            #      per-row transposes + kv matmuls ----
            ms = [msp.tile([128, 66, 128], BF16, tag=f"ms{t}", name=f"ms{t}")
                  for t in range(3)]
            with tc.tile_pool(name="dps", bufs=2, space="PSUM") as dps, \
                 tc.tile_pool(name="kps", bufs=1, space="PSUM") as kps, \
                 tc.tile_pool(name="tpp", bufs=4, space="PSUM") as tpp, \
                 tc.tile_pool(name="mtp", bufs=8) as mtp:
                kvT = [kps.tile([128, 144], F32, tag=f"kvT{h}", name=f"kvT{h}")
                       for h in range(2)]

                def kv_row(r):
                    # all 6 row transposes on PE (identity transpose-matmul);
                    # psum->sbuf copies alternate DVE/ACT
                    mT = mtp.tile([128, 768], BF16, tag="mT", name="mT")
                    for i, (src, dst0) in enumerate(
                            [(qkv[0][:, 1 + r, :], 0),
                             (ms[0][:, 1 + r, :], 384),
                             (qkv[1][:, 1 + r, :], 128),
                             (ms[1][:, 1 + r, :], 512),
                             (qkv[2][:, 1 + r, :], 256),
                             (ms[2][:, 1 + r, :], 640)]):
                        tp = tpp.tile([128, 128], BF16, tag="tp", name="tp")
                        nc.tensor.transpose(tp, src, ident)
                        if (i + r) % 2 == 0:
                            nc.vector.tensor_copy(
                                out=mT[:, dst0:dst0 + 128], in_=tp)
                        else:
                            nc.scalar.activation(
                                out=mT[:, dst0:dst0 + 128], in_=tp,
                                func=AF.Copy)
                    mg = mT.rearrange("p (g c) -> p g c", c=24)
                    kc = mtp.tile([128, 256], BF16, tag="kc", name="kc")
                    nc.vector.tensor_copy(
                        out=kc.rearrange("p (g e) -> p g e", e=8),
                        in_=mg[:, :, 8:16])
                    vc = vcs[r % 2]
                    nc.vector.tensor_copy(
                        out=vc.rearrange("p h g e -> p (h g) e")[:, :, 0:8],
                        in_=mg[:, :, 16:24])
                    for h in range(2):
                        kcols = kc[:, 128 * h:128 * h + 128]
                        nc.tensor.matmul(kvT[h], kcols, vc[:, h, :, :],
                                         start=(r == 0), stop=(r == 63))

                rdone = 0
                for ci, (c0, cn) in enumerate(_subs(66)):
                    for t in range(3):
                        ps = dps.tile([128, 512], F32, tag="dps", name="dps")
                        q8 = qkv8[t]
                        for p_, (dyA, dxA, dyB, dxB, _ra) in enumerate(
                                PAIRS5):
                            dlt = (dyB - dyA) * 132 + (dxB - dxA)
                            win = q8[:, c0 + dyA:c0 + dyA + cn,
                                     dxA:dxA + 128]
                            rhs = bass.AP(
                                tensor=win.tensor, offset=win.offset,
                                ap=[list(win.ap[0]), [dlt, 2],
                                    list(win.ap[1]), list(win.ap[2])])
                            w8 = dwW[:, (t * NP5 + p_) * 256:
                                     (t * NP5 + p_) * 256 + 256].rearrange(
                                "p (k m) -> p k m", k=2)
                            nc.tensor.matmul(ps[:, 0:cn * 128], w8, rhs,
                                             start=(p_ == 0),
                                             stop=(p_ == NP5 - 1),
                                             perf_mode=DR)
                        nc.scalar.activation(
                            out=ms[t][:, c0:c0 + cn, :],
                            in_=ps[:, 0:cn * 128].rearrange(
                                "p (r w) -> p r w", w=128),
                            func=AF.Copy, scale=1.0 / S5)
                        nc.vector.tensor_scalar_max(
                            out=ms[t][:, c0:c0 + cn, :],
                            in0=ms[t][:, c0:c0 + cn, :],
                            scalar1=cm[:, t:t + 1])
                    lim = min(64, c0 + cn - 1)
                    while rdone < lim:
                        kv_row(rdone)
                        rdone += 1
                while rdone < 64:
                    kv_row(rdone)
                    rdone += 1
                with nc.allow_low_precision(reason="bf16 kv exchange"):
                    for h in range(2):
                        nc.vector.tensor_copy(
                            out=comp[:, 144 * h:144 * h + 144], in_=kvT[h])
            dwstack.close()
            if dbg and dbg.startswith("ms"):
                ti = int(dbg[2])
                nc.sync.dma_start(
                    out=d["dbg"][:, 0:66 * 128].rearrange(
                        "p (r w) -> p r w", w=128), in_=ms[ti])
            if dbg and dbg.startswith("qm"):
                ti = int(dbg[2])
                nc.sync.dma_start(
                    out=d["dbg"][:, 0:66 * 128].rearrange(
                        "p (r w) -> p r w", w=128), in_=qkv[ti])

            # ---- P5: AllReduce + scatter ----
            nc.sync.dma_start(out=cc_in[:], in_=comp)
            if sim:
                nc.sync.dma_start(out=cc_out[:], in_=cc_in[:])
            else:
                nc.gpsimd.collective_compute(
                    "AllReduce", Alu.add,
                    replica_groups=[[0, 1], [2, 3], [4, 5], [6, 7]],
                    ins=[cc_in.opt()], outs=[cc_out.opt()])
            for g in range(32):
                a, gl9 = g // 12, g % 12
                h, gl = g // 16, g % 16
                S, row0 = q_chan(g, 0) // 128, q_chan(g, 0) % 128
                L = lhsT_att[(a, S)].rearrange("p (dd gl) -> p dd gl", gl=12)
                eng = (nc.sync, nc.scalar, nc.gpsimd)[g % 3]
                eng.dma_start(
                    out=L[row0:row0 + 8, 0:9, gl9:gl9 + 1],
                    in_=cc_out[8 * gl:8 * gl + 8,
                               144 * h + 9 * gl:144 * h + 9 * gl + 9])
            # ---- P6+P7 fused: att9 + division + proj + residual per chunk ----
            # denominator reciprocal on ACT via exp(-ln(x + eps)): the exp/ln
            # table set also holds copy/identity/relu so no table reloads; the
            # DVE 8-pass iterative reciprocal is gone entirely.
            with tc.tile_pool(name="aps", bufs=2, space="PSUM") as aps, \
                 tc.tile_pool(name="bps", bufs=1, space="PSUM") as bpsp, \
                 tc.tile_pool(name="jps", bufs=1, space="PSUM") as jpsp, \
                 tc.tile_pool(name="dnp", bufs=3) as dnp:
                epsb = dnp.tile([128, 1], F32, tag="epsb", name="epsb",
                                bufs=1)
                nc.vector.memset(epsb, 1e-15)

                def att_rhs(S, c0, cn):
                    if S < 3:
                        return qkv[S][:, c0:c0 + cn, :]
                    return ms[S - 3][:, c0:c0 + cn, :]

                for ci, (c0, cn) in enumerate(_subs(66)):
                    cw = cn * 128
                    psl = aps.tile([108, 3 * 512], F32, tag="aps", name="aps")
                    for a in range(3):
                        srcs = ATT_SRCS[a]
                        for i, S in enumerate(srcs):
                            nc.tensor.matmul(psl[:, a * 512:a * 512 + cw],
                                             lhsT_att[(a, S)],
                                             att_rhs(S, c0, cn), start=(i == 0),
                                             stop=(i == len(srcs) - 1))
                    lnden = dnp.tile([12, 3 * 512], F32, tag="lnden",
                                     name="lnden")
                    nc.scalar.activation(out=lnden, in_=psl[96:108, :],
                                         func=AF.Ln, bias=epsb[0:12, 0:1],
                                         scale=1.0)
                    rden = dnp.tile([12, 3 * 512], BF16, tag="rden",
                                    name="rden")
                    with nc.allow_low_precision(reason="den recip to bf16"):
                        nc.scalar.activation(out=rden, in_=lnden, func=AF.Exp,
                                             scale=-1.0)
                    jp = jpsp.tile([128, 512], F32, tag="jps", name="jps")
                    for a in range(3):
                        bp = bpsp.tile([96, 512], F32, tag="bps", name="bps")
                        nc.tensor.matmul(bp, brd[0:12, :],
                                         rden[:, a * 512:a * 512 + 512],
                                         start=True, stop=True)
                        recb = dnp.tile([96, 512], BF16, tag="recb",
                                        name="recb")
                        with nc.allow_low_precision(reason="recb bf16"):
                            if a == 1:
                                nc.scalar.activation(out=recb, in_=bp,
                                                     func=AF.Copy)
                            else:
                                nc.vector.tensor_copy(out=recb, in_=bp)
                        attc = dnp.tile([96, 512], BF16, tag="attc",
                                        name="attc")
                        with nc.allow_low_precision(reason="att to bf16"):
                            nc.vector.tensor_mul(
                                out=attc, in0=psl[0:96, a * 512:a * 512 + 512],
                                in1=recb)
                        nc.tensor.matmul(jp,
                                         pjw[0:96, a * 128:a * 128 + 128],
                                         attc,
                                         start=(a == 0), stop=(a == 2))
                    aB = dnp.tile([128, 512], BF16, tag="aB", name="aB")
                    nc.scalar.activation(out=aB, in_=jp,
                                         func=AF.Identity, bias=pjb[:, 0:1],
                                         scale=1.0)
                    nc.vector.tensor_add(
                        out=attf[:, c0:c0 + cn, :],
                        in0=aB[:, 0:cw].rearrange("p (r w) -> p r w", w=128),
                        in1=xr[:, c0 + 3:c0 + 3 + cn, 1:129])

        if dbg == "attf":
            nc.sync.dma_start(
                out=d["dbg"][:, 0:66 * 128].rearrange(
                    "p (r w) -> p r w", w=128), in_=attf)
        # ---- P8: mb1 + hswish -> h1 fp8 (SBUF resident) ----
        with tc.tile_pool(name="h1p", bufs=1) as h1p, \
             tc.tile_pool(name="lwp", bufs=1) as lwp:
            dw3W = lwp.tile([128, 6 * NP3 * 256], F8, tag="dw3W", name="dw3W")
            nc.sync.dma_start(out=dw3W, in_=d["dw3W"])
            h1 = []
            for t in range(6):
                ht = h1p.tile([128, 66, 130], F8, tag=f"h1{t}", name=f"h1{t}")
                nc.vector.memset(ht[:, 0:1, :], 0.0)
                nc.vector.memset(ht[:, 1:66, 0:1], 0.0)
                nc.vector.memset(ht[:, 1:66, 129:130], 0.0)
                h1.append(ht)
            with tc.tile_pool(name="m1ps", bufs=2, space="PSUM") as m1ps, \
                 tc.tile_pool(name="hwp", bufs=2) as hwp:
                for (r0, nr) in _row_groups(65, base=1):
                    for t in range(6):
                        ps = m1ps.tile([128, 16 * 128], F32, tag="m1ps", name="m1ps")
                        for (sr, sn) in _subs(nr):
                            nc.tensor.matmul(
                                ps[:, sr * 128:(sr + sn) * 128],
                                m1w[:, t * 128:t * 128 + 128],
                                attf[:, r0 + sr:r0 + sr + sn, :],
                                start=True, stop=True)
                        pw_ = ps[:, 0:nr * 128]
                        # 6*hswish(x) = min(relu(x+3),6)*x in 2 DVE ops; the
                        # 1/6 is folded into the host dw3 weights
                        th = hwp.tile([128, 16 * 128], BF16, tag="th", name="th")
                        xh = hwp.tile([128, 16 * 128], BF16, tag="xh", name="xh")
                        nc.scalar.activation(out=xh[:, 0:nr * 128], in_=pw_,
                                             func=AF.Identity,
                                             bias=m1b[:, t:t + 1], scale=1.0)
                        nc.vector.tensor_scalar(
                            out=th[:, 0:nr * 128], in0=xh[:, 0:nr * 128],
                            scalar1=3.0, scalar2=0.0,
                            op0=Alu.add, op1=Alu.max)
                        if t % 2 == 0:
                            with nc.allow_low_precision(reason="h1 fp8"):
                                nc.vector.scalar_tensor_tensor(
                                    out=h1[t][:, r0:r0 + nr, 1:129],
                                    in0=th[:, 0:nr * 128].rearrange(
                                        "p (r w) -> p r w", w=128),
                                    scalar=6.0,
                                    in1=xh[:, 0:nr * 128].rearrange(
                                        "p (r w) -> p r w", w=128),
                                    op0=Alu.min, op1=Alu.mult)
                        else:
                            hb = hwp.tile([128, 16 * 128], BF16, tag="hb",
                                          name="hb")
                            nc.vector.scalar_tensor_tensor(
                                out=hb[:, 0:nr * 128], in0=th[:, 0:nr * 128],
                                scalar=6.0, in1=xh[:, 0:nr * 128],
                                op0=Alu.min, op1=Alu.mult)
                            with nc.allow_low_precision(reason="h1 fp8"):
                                nc.scalar.activation(
                                    out=h1[t][:, r0:r0 + nr, 1:129],
                                    in_=hb[:, 0:nr * 128].rearrange(
                                        "p (r w) -> p r w", w=128),
                                    func=AF.Copy)

            # ---- P9: dw3 (fp8 DoubleRow) + hswish + mb3 + final adds ----
            with tc.tile_pool(name="d3ps", bufs=2, space="PSUM") as d3ps, \
                 tc.tile_pool(name="m3ps", bufs=2, space="PSUM") as m3ps, \
                 tc.tile_pool(name="h2p", bufs=2) as h2p, \
                 tc.tile_pool(name="osp", bufs=2) as osp:
                m3w8 = lwp.tile([128, 3 * 256], F8, tag="m3w8", name="m3w8")
                nc.scalar.dma_start(out=m3w8, in_=d["m3w8"])
                for q in range(8):   # 8-row half-bands, out rows lr 8q..8q+8
                    ps = m3ps.tile([128, 8 * 128], F32, tag="m3ps", name="m3ps")
                    h2s = h2p.tile([128, 6, 8 * 128], F8, tag="h2s",
                                   name="h2s")
                    for t in range(6):
                        dp = d3ps.tile([128, 8 * 128], F32, tag="d3ps", name="d3ps")
                        hh = h1[t]
                        for (sr, sn) in _subs(8):
                            for p_, (dyA, dxA, dyB, dxB, _ra) in enumerate(
                                    PAIRS3):
                                dlt = (dyB - dyA) * 130 + (dxB - dxA)
                                win = hh[:, 8 * q + sr + dyA:
                                         8 * q + sr + dyA + sn,
                                         dxA:dxA + 128]
                                rhs = bass.AP(
                                    tensor=win.tensor, offset=win.offset,
                                    ap=[list(win.ap[0]), [dlt, 2],
                                        list(win.ap[1]), list(win.ap[2])])
                                w8 = dw3W[:, (t * NP3 + p_) * 256:
                                          (t * NP3 + p_) * 256 + 256].rearrange(
                                    "p (k m) -> p k m", k=2)
                                nc.tensor.matmul(
                                    dp[:, sr * 128:(sr + sn) * 128], w8, rhs,
                                    start=(p_ == 0), stop=(p_ == NP3 - 1),
                                    perf_mode=DR)
                        th = h2p.tile([128, 8 * 128], BF16, tag="th2", name="th2")
                        xh = h2p.tile([128, 8 * 128], BF16, tag="xh2", name="xh2")
                        nc.scalar.activation(out=xh, in_=dp, func=AF.Identity,
                                             bias=m2b[:, t:t + 1], scale=1.0 / S3)
                        nc.vector.tensor_scalar(
                            out=th, in0=xh, scalar1=3.0, scalar2=0.0,
                            op0=Alu.add, op1=Alu.max)
                        with nc.allow_low_precision(reason="h2 fp8"):
                            nc.vector.scalar_tensor_tensor(
                                out=h2s[:, t, :], in0=th, scalar=6.0, in1=xh,
                                op0=Alu.min, op1=Alu.mult)
                    for pr in range(3):
                        for (sr, sn) in _subs(8):
                            win = h2s[:, 2 * pr, sr * 128:(sr + sn) * 128]
                            rhs = bass.AP(
                                tensor=win.tensor, offset=win.offset,
                                ap=[list(win.ap[0]), [1024, 2],
                                    list(win.ap[1])])
                            w8 = m3w8[:, pr * 256:pr * 256 + 256].rearrange(
                                "p (k m) -> p k m", k=2)
                            nc.tensor.matmul(
                                ps[:, sr * 128:(sr + sn) * 128], w8, rhs,
                                start=(pr == 0), stop=(pr == 2),
                                perf_mode=DR)
                    o1 = osp.tile([128, 8 * 128], F32, tag="o1", name="o1")
                    nc.scalar.activation(out=o1, in_=ps, func=AF.Identity,
                                         bias=m3b[:, 0:1], scale=1.0 / S3M)
                    nc.vector.tensor_add(
                        out=o1, in0=o1,
                        in1=attf[:, 8 * q + 1:8 * q + 9, :].rearrange(
                            "p r w -> p (r w)"))
                    nc.sync.dma_start(out=d["out"][:, 8 * q:8 * q + 8, :],
                                      in_=o1.rearrange("p (r w) -> p r w", w=128))


# ====================== host side ======================

def _prep_shared(inp):
    f32 = np.float32
    out = {}
    pw = inp["agg_pw_w"][:, :, 0, 0]          # [384, 8]
    w5 = inp["agg_dw_w"][:, 0, :, :]          # [384, 5, 5]
    w3 = inp["mb2_w"][:, 0, :, :]             # [768, 3, 3]
    for s in (0, 1):
        w = {}
        wc = np.zeros((128, 3 * NP3 * 256), f32)
        for j, cw in enumerate((inp["wq"], inp["wk"], inp["wv"])):
            def c_mat(dy, dx):
                dyy = 2 - dy if s == 1 else dy
                return cw[:, :, dyy, dx].T * SC
            for p_, (dyA, dxA, dyB, dxB, realA) in enumerate(PAIRS3):
                k = (j * NP3 + p_) * 256
                if realA:
                    wc[:, k:k + 128] = c_mat(dyA, dxA)
                wc[:, k + 128:k + 256] = c_mat(dyB, dxB)
        w["wc"] = wc.astype(F8NP)
        w["cb"] = np.stack([inp["bq"], inp["bk"], inp["bv"]], 1).astype(f32)
        m = np.arange(384)
        w["cm"] = np.where((m % 24) < 16, 0.0, -1e9).astype(f32).reshape(3, 128).T.copy()
        # fused dw5x5 + grouped pw block-diag weights, fp8 DoubleRow pairs
        def dw5_mat(t, dy, dx):
            dyy = 4 - dy if s == 1 else dy
            M = np.zeros((128, 128), f32)
            for b in range(16):
                i0 = 8 * b
                blk = (w5[128 * t + i0:128 * t + i0 + 8, dyy, dx][:, None]
                       * pw[128 * t + i0:128 * t + i0 + 8, :].T)
                M[i0:i0 + 8, i0:i0 + 8] = blk
            return M * S5
        dwW = np.zeros((128, 3 * NP5 * 256), f32)
        for t in range(3):
            for p_, (dyA, dxA, dyB, dxB, realA) in enumerate(PAIRS5):
                k = (t * NP5 + p_) * 256
                if realA:
                    dwW[:, k:k + 128] = dw5_mat(t, dyA, dxA)
                dwW[:, k + 128:k + 256] = dw5_mat(t, dyB, dxB)
        w["dwW"] = dwW.astype(F8NP)
        # dw3 diagonal weights, fp8 DoubleRow vertical pairs
        def dw3_mat(t, dy, dx):
            dyy = 2 - dy if s == 1 else dy
            M = np.zeros((128, 128), f32)
            M[np.arange(128), np.arange(128)] = \
                w3[128 * t:128 * t + 128, dyy, dx] * (S3 / 6.0)
            return M
        dw3W = np.zeros((128, 6 * NP3 * 256), f32)
        for t in range(6):
            for p_, (dyA, dxA, dyB, dxB, realA) in enumerate(PAIRS3):
                k = (t * NP3 + p_) * 256
                if realA:
                    dw3W[:, k:k + 128] = dw3_mat(t, dyA, dxA)
                dw3W[:, k + 128:k + 256] = dw3_mat(t, dyB, dxB)
        w["dw3W"] = dw3W.astype(F8NP)
        # one-hot broadcast for denominators: rows 96+gl9 -> out col o (gl9=o%12)
        brd = np.zeros((128, 96), f32)
        o = np.arange(96)
        brd[o % 12, o] = 1.0
        w["brd"] = brd.astype(BF)
        s1 = inp["bn1_g"] / np.sqrt(inp["bn1_v"] + BN_EPS)
        b1 = inp["bn1_b"] - inp["bn1_m"] * s1
        Wp = inp["attn_proj_w"][:, :, 0, 0] * s1[:, None]
        pjw = np.zeros((128, 3 * 128), f32)
        for g in range(32):
            a, gl9 = g // 12, g % 12
            for dd in range(8):
                pjw[12 * dd + gl9, a * 128:a * 128 + 128] = Wp[:, 8 * g + dd]
        w["pjw"] = pjw.astype(BF)
        w["pjb"] = b1.reshape(128, 1).astype(f32)
        m1w = np.zeros((128, 6 * 128), f32)
        for t in range(6):
            m1w[:, t * 128:t * 128 + 128] = inp["mb1_w"][128 * t:128 * t + 128, :, 0, 0].T
        w["m1w"] = m1w.astype(BF)
        w["m1b"] = inp["mb1_b"].reshape(6, 128).T.copy().astype(f32)
        w["m2b"] = inp["mb2_b"].reshape(6, 128).T.copy().astype(f32)
        s2 = inp["bn2_g"] / np.sqrt(inp["bn2_v"] + BN_EPS)
        b2 = inp["bn2_b"] - inp["bn2_m"] * s2
        W3 = inp["mb3_w"][:, :, 0, 0] * s2[:, None]
        m3w8 = np.zeros((128, 3 * 256), f32)
        for t in range(6):
            half = 128 * (t % 2)
            k = (t // 2) * 256 + half
            m3w8[:, k:k + 128] = \
                W3[:, 128 * t:128 * t + 128].T * (S3M / 6.0)
        w["m3w8"] = m3w8.astype(F8NP)
        w["m3b"] = b2.reshape(128, 1).astype(f32)
        w["idt"] = np.eye(128, dtype=f32).astype(BF)
        out[s] = w
    return out


def _prep_core(inp, b, s):
    f32 = np.float32
    ref = inp["ref_features"][b]
    oth = inp["other_features"][b]
    if s == 1:
        ref = ref[:, ::-1, :]
        oth = oth[:, ::-1, :]
    xr = np.zeros((128, 72, 130), f32)
    xo = np.zeros((128, 72, 130), f32)
    xr[:, 4:72, 1:129] = ref[:, 0:68, :]
    xo[:, 4:72, 1:129] = oth[:, 0:68, :]
    return {"xr": xr.astype(BF), "xr8": xr.astype(F8NP),
            "xo8": xo.astype(F8NP)}


def make_in_maps(inp):
    ws = _prep_shared(inp)
    in_maps = []
    for c in range(NCORES):
        b, s = c // 2, c % 2
        m = dict(ws[s])
        m.update(_prep_core(inp, b, s))
        in_maps.append(m)
    return in_maps


def kernel(**inputs):
    inp = {k: np.asarray(v) for k, v in inputs.items()}
    if "nc" not in _CACHE:
        _CACHE["nc"] = build_program()
    nc = _CACHE["nc"]
    in_maps = make_in_maps(inp)
    res = bass_utils.run_bass_kernel_spmd(nc, in_maps,
                                          core_ids=list(range(NCORES)))
    out = np.zeros((4, 128, 128, 128), np.float32)
    for c in range(NCORES):
        b, s = c // 2, c % 2
        o = res.results[c]["out"]
        if s == 1:
            o = o[:, ::-1, :]
        out[b, :, 64 * s:64 * s + 64, :] = o
    return out

